# revision 1
# baseline (speedup 1.0000x reference)
"""RGCN (2x hetero GraphConv + mean-pool + MLP) on 8 TRN2 NeuronCores — v2.

Nodes are dst-sharded 12500/core. Per layer, each core aggregates its dst
rows: edges are sorted by (dst-block, src-group) with all 4 relations
merged; source features are fetched per-edge with SWDGE dma_gather (int16
indices, 4 src groups of 25000 rows) and reduced into per-dst-block PSUM
accumulators [H, 4*128] (f32) by one-hot PE matmuls — column r*128+dslot
carries edge weight selection, so no scatter DMA exists at all. Relation
weights W_r then contract the accumulator slices; layer 1 applies
relu+bias and stores its shard (AllGather -> full table for layer 2);
layer 2 feeds mean-pooling (matmul with a host-built graph-assignment
matrix), AllReduce, and the small MLP head.

The instruction stream is identical on all 8 cores (SPMD); per-core
variation lives in input tensors (indices, one-hot metadata, graph
assignment). Host-side numpy only computes graph-structure metadata
(degrees/index layouts) and dtype/layout staging of inputs.
"""

import numpy as np
from ml_dtypes import bfloat16

import concourse.bass as bass
import concourse.bacc as bacc
import concourse.mybir as mybir
import concourse.tile as tile
from concourse import bass_utils
from concourse.masks import make_identity

F32 = mybir.dt.float32
F16 = mybir.dt.float16
BF16 = mybir.dt.bfloat16
I16 = mybir.dt.int16
I32 = mybir.dt.int32

# problem constants (hardcoded per spec)
N, E, NREL, G, IN, H, C = 100000, 400000, 4, 64, 64, 128, 2
CORES = 8
SHARD = N // CORES            # 12500
NBLK = (SHARD + 127) // 128   # 98
GRP = 25000                   # src rows per gather group (int16 idx limit)
NGRP = N // GRP               # 4
OW = NREL * 128               # one-hot width (relation-merged): 512
SBS = 4                       # dst blocks per superblock (PSUM banks)
KMAX = 32                     # max 128-edge chunks per gather call


# ---------------------------------------------------------------------------
# host-side planning: pure graph-structure metadata (indices, degrees, layout)
# ---------------------------------------------------------------------------

def _plan(src, dst, graph_ids):
    src = np.asarray(src).astype(np.int64)
    dst = np.asarray(dst).astype(np.int64)
    gid = np.asarray(graph_ids).astype(np.int64)

    # folded normalization: w_e = rsqrt(deg_in[dst]) * rsqrt(deg_out[src])
    w_all = np.empty((NREL, E), np.float32)
    for r in range(NREL):
        do = np.maximum(np.bincount(src[r], minlength=N), 1.0)
        di = np.maximum(np.bincount(dst[r], minlength=N), 1.0)
        w_all[r] = (1.0 / np.sqrt(do[src[r]]) / np.sqrt(di[dst[r]])).astype(np.float32)

    # per (core, block, group) cells; relations merged inside a cell
    cells = {}
    cnt = np.zeros((CORES, NBLK, NGRP), np.int64)
    for c in range(CORES):
        for b in range(NBLK):
            for g in range(NGRP):
                cells[(c, b, g)] = []
        for r in range(NREL):
            local = dst[r] - c * SHARD
            m = (local >= 0) & (local < SHARD)
            es = np.nonzero(m)[0]
            loc = local[es]
            b_arr = loc // 128
            g_arr = src[r][es] // GRP
            key = b_arr * NGRP + g_arr
            order = np.argsort(key, kind="stable")
            es, loc, b_arr, g_arr = es[order], loc[order], b_arr[order], g_arr[order]
            keys = b_arr * NGRP + g_arr
            bounds = np.searchsorted(keys, np.arange(NBLK * NGRP + 1))
            for cell in range(NBLK * NGRP):
                s0, s1 = bounds[cell], bounds[cell + 1]
                if s0 == s1:
                    continue
                b, g = divmod(cell, NGRP)
                ee = es[s0:s1]
                cells[(c, b, g)].append((
                    (src[r][ee] - g * GRP).astype(np.int16),
                    (r * 128 + loc[s0:s1] - b * 128).astype(np.float32),
                    w_all[r][ee].astype(np.float32),
                ))
                cnt[c, b, g] += s1 - s0

    # SPMD-uniform chunk counts: global max per cell, capped at CAP chunks;
    # overflow edges route to per-(superblock, group) residual chunks whose
    # one-hots are built per dst block (mask via dsl = -1)
    CAP = 4
    nch = np.minimum(-(-cnt.max(axis=0) // 128), CAP)   # [NBLK, NGRP]
    resid_cnt = np.maximum(cnt - nch[None, :, :] * 128, 0)  # [CORES, NBLK, NGRP]

    sbs = []
    q = 0       # chunk slots (st/WC columns)
    dcol = 0    # DSL columns (one-hots)
    col = 0     # idx int16 columns
    for s0 in range(0, NBLK, SBS):
        blocks = list(range(s0, min(s0 + SBS, NBLK)))
        rc = {g: int(max(-(-resid_cnt[c, blocks, g].sum() // 128)
                         for c in range(CORES)))
              for g in range(NGRP)}
        # chunk stream per g: ("main", b) then ("resid",)
        mm_all = []     # flat matmul list for start/stop flags
        calls = []
        for g in range(NGRP):
            descs = [("main", b) for b in blocks
                     for _ in range(int(nch[b, g]))]
            descs += [("resid",)] * rc[g]
            for i0 in range(0, len(descs), KMAX):
                part = descs[i0:i0 + KMAX]
                kc = len(part)
                ohs = []    # (chunk_i, dsl col offset rel to d0)
                mms = []    # (chunk_i, oh_i, b)
                nd = 0
                for i, d in enumerate(part):
                    if d[0] == "main":
                        ohs.append((i, nd))
                        mms.append((i, len(ohs) - 1, d[1]))
                        nd += 1
                    else:
                        for bi, b in enumerate(blocks):
                            ohs.append((i, nd + bi))
                            mms.append((i, len(ohs) - 1 , b))
                        nd += len(blocks)
                call = dict(g=g, kc=kc, q0=q, d0=dcol, col0=col,
                            part=part, ohs=ohs, mms=mms)
                calls.append(call)
                mm_all.extend((len(calls) - 1, j) for j in range(len(mms)))
                q += kc
                dcol += nd
                col += kc * 8
        # start/stop flags per block over the sb's matmul order
        occ = {}
        for ci, call in enumerate(calls):
            for j, (i, oi, b) in enumerate(call["mms"]):
                occ.setdefault(b, []).append((ci, j))
        tb = {b: len(occ.get(b, [])) for b in blocks}
        for ci, call in enumerate(calls):
            flags = []
            for j, (i, oi, b) in enumerate(call["mms"]):
                first = occ[b][0] == (ci, j)
                last = occ[b][-1] == (ci, j)
                flags.append((i, oi, b, first, last))
            call["mms"] = flags
        sbs.append(dict(blocks=blocks, calls=calls, tb=tb))
    NCH = q
    NDCOL = dcol
    NIDXCOL = col

    # one-hot column ids split into two bf16-exact halves: lo compares
    # dsl against 0..255, hi compares dsl-256 (so every integer lands in
    # bf16's exact range; out-of-half values can never match 0..255)
    IDX = np.zeros((CORES, 16, NIDXCOL), np.int16)
    DSL = np.full((CORES, 128, NDCOL), -1.0, bfloat16)
    DSH = np.full((CORES, 128, NDCOL), -257.0, bfloat16)
    WC = np.zeros((CORES, 128, NCH), np.float32)
    for c in range(CORES):
        for sb in sbs:
            blocks = sb["blocks"]
            # cell cursors (capped) + residual pools per group
            cur = {}
            for b in blocks:
                for g in range(NGRP):
                    parts = cells[(c, b, g)]
                    if parts:
                        si = np.concatenate([p[0] for p in parts])
                        dl = np.concatenate([p[1] for p in parts])
                        wv = np.concatenate([p[2] for p in parts])
                    else:
                        si = np.zeros(0, np.int16)
                        dl = np.zeros(0, np.float32)
                        wv = np.zeros(0, np.float32)
                    cur[(b, g)] = [si, dl, wv, 0]
            resid = {g: None for g in range(NGRP)}
            for g in range(NGRP):
                ss, dd, ww, bb = [], [], [], []
                for b in blocks:
                    quota = int(nch[b, g]) * 128
                    si, dl, wv, _ = cur[(b, g)]
                    if len(si) > quota:
                        ss.append(si[quota:])
                        dd.append(dl[quota:])
                        ww.append(wv[quota:])
                        bb.append(np.full(len(si) - quota, b, np.int64))
                if ss:
                    resid[g] = [np.concatenate(ss), np.concatenate(dd),
                                np.concatenate(ww), np.concatenate(bb), 0]
            for call in sb["calls"]:
                g = call["g"]
                for i, d in enumerate(call["part"]):
                    qq = call["q0"] + i
                    nn0 = i * 128
                    if d[0] == "main":
                        b = d[1]
                        si, dl, wv, pos = cur[(b, g)]
                        take = min(128, int(nch[b, g]) * 128 - pos,
                                   len(si) - pos)
                        if take > 0:
                            sl = slice(pos, pos + take)
                            nn = nn0 + np.arange(take)
                            IDX[c, nn % 16, call["col0"] + nn // 16] = si[sl]
                            # main chunk's one-hot is its own dsl column
                            oi = [o for o in call["ohs"] if o[0] == i]
                            DSL[c, :take, call["d0"] + oi[0][1]] = dl[sl]
                            DSH[c, :take, call["d0"] + oi[0][1]] = dl[sl] - 256.0
                            WC[c, :take, qq] = wv[sl]
                            cur[(b, g)][3] = pos + take
                    else:
                        rp = resid[g]
                        if rp is None:
                            continue
                        si, dl, wv, bb, pos = rp
                        take = min(128, len(si) - pos)
                        if take > 0:
                            sl = slice(pos, pos + take)
                            nn = nn0 + np.arange(take)
                            IDX[c, nn % 16, call["col0"] + nn // 16] = si[sl]
                            WC[c, :take, qq] = wv[sl]
                            # per-block masked dsl columns
                            ohs_i = [o for o in call["ohs"] if o[0] == i]
                            for bi, b in enumerate(blocks):
                                m = bb[sl] == b
                                DSL[c, :take, call["d0"] + ohs_i[bi][1]] = \
                                    np.where(m, dl[sl], -1.0)
                                DSH[c, :take, call["d0"] + ohs_i[bi][1]] = \
                                    np.where(m, dl[sl] - 256.0, -257.0)
                            rp[4] = pos + take

    # graph assignment matrix with 1/count folded in
    cnt_g = np.maximum(np.bincount(gid, minlength=G), 1.0)
    gmat = np.zeros((CORES, NBLK * 128, G), np.float32)
    for c in range(CORES):
        ids = gid[c * SHARD:(c + 1) * SHARD]
        gmat[c, np.arange(SHARD), ids] = 1.0 / cnt_g[ids]

    return dict(sbs=sbs, NCH=NCH, NDCOL=NDCOL, NIDXCOL=NIDXCOL,
                IDX=np.tile(IDX, (1, 8, 1)),
                DSL=DSL, DSH=DSH, WC=WC,
                gmat=gmat.astype(bfloat16))


# ---------------------------------------------------------------------------
# device program
# ---------------------------------------------------------------------------

def _build(plan):
    sbs = plan["sbs"]
    NCH = plan["NCH"]
    NDCOL = plan["NDCOL"]
    NIDXCOL = plan["NIDXCOL"]

    nc = bacc.Bacc(None, target_bir_lowering=False, num_devices=CORES)

    p = {}
    p["xTs"] = nc.declare_dram_parameter("xTs", [IN + 1, SHARD], BF16, isOutput=False)
    p["W65"] = nc.declare_dram_parameter("W65", [IN + 1, H], BF16, isOutput=False)
    p["Wl1"] = nc.declare_dram_parameter("Wl1", [NREL, H, H], BF16, isOutput=False)
    p["Wl2"] = nc.declare_dram_parameter("Wl2", [NREL, H, H], BF16, isOutput=False)
    p["Wm1"] = nc.declare_dram_parameter("Wm1", [H, H], BF16, isOutput=False)
    p["Wm2"] = nc.declare_dram_parameter("Wm2", [H, H], BF16, isOutput=False)
    p["Wm3"] = nc.declare_dram_parameter("Wm3", [H, C], BF16, isOutput=False)
    p["B1"] = nc.declare_dram_parameter("B1", [H, 1], F32, isOutput=False)
    p["B2"] = nc.declare_dram_parameter("B2", [H, 1], F32, isOutput=False)
    p["bm1"] = nc.declare_dram_parameter("bm1", [H, 1], F32, isOutput=False)
    p["bm2"] = nc.declare_dram_parameter("bm2", [H, 1], F32, isOutput=False)
    p["bm3"] = nc.declare_dram_parameter("bm3", [C, 1], F32, isOutput=False)
    p["IDX"] = nc.declare_dram_parameter("IDX", [128, NIDXCOL], I16, isOutput=False)
    p["DSL"] = nc.declare_dram_parameter("DSL", [128, NDCOL], BF16, isOutput=False)
    p["DSH"] = nc.declare_dram_parameter("DSH", [128, NDCOL], BF16, isOutput=False)
    p["WC"] = nc.declare_dram_parameter("WC", [128, NCH], F32, isOutput=False)
    p["gmat"] = nc.declare_dram_parameter("gmat", [NBLK * 128, G], BF16, isOutput=False)
    out_ext = nc.declare_dram_parameter("out", [C, G], F32, isOutput=True)

    h0_shard = nc.dram_tensor("h0_shard", [SHARD, H], BF16)
    h0_full = nc.dram_tensor("h0_full", [N, H], BF16, addr_space="Shared")
    h1_shard = nc.dram_tensor("h1_shard", [SHARD, H], BF16)
    h1_full = nc.dram_tensor("h1_full", [N, H], BF16, addr_space="Shared")
    pool_in = nc.dram_tensor("pool_in", [H, G], F32)
    pool_out = nc.dram_tensor("pool_out", [H, G], F32, addr_space="Shared")

    rg = [list(range(CORES))]

    with tile.TileContext(nc) as tc:
        with (
            tc.tile_pool(name="const", bufs=1) as cpool,
            tc.tile_pool(name="idx", bufs=3) as ipool,
            tc.tile_pool(name="meta", bufs=3) as mpool,
            tc.tile_pool(name="gath", bufs=3) as dpool,
            tc.tile_pool(name="scal", bufs=2) as spool,
            tc.tile_pool(name="oneh", bufs=2) as opool,
            tc.tile_pool(name="aggs", bufs=2) as apool,
            tc.tile_pool(name="work", bufs=4) as wpool,
            tc.tile_pool(name="pa", bufs=1, space="PSUM") as pa,
            tc.tile_pool(name="po", bufs=2, space="PSUM") as po,
            tc.tile_pool(name="pb", bufs=1, space="PSUM") as pb,
            tc.tile_pool(name="pp", bufs=1, space="PSUM") as pp,
        ):
            # ---- constants
            id_f32 = cpool.tile([128, 128], F32)
            make_identity(nc, id_f32[:])
            id_bf = cpool.tile([128, 128], BF16)
            nc.vector.tensor_copy(id_bf[:], id_f32[:])

            iota_i = cpool.tile([128, 256], I32)
            nc.gpsimd.iota(iota_i[:], pattern=[[1, 256]], base=0,
                           channel_multiplier=0)
            iota_f = cpool.tile([128, 256], BF16)
            nc.vector.tensor_copy(iota_f[:], iota_i[:])

            w65 = cpool.tile([IN + 1, H], BF16)
            nc.sync.dma_start(w65[:], p["W65"][:, :])
            wl = {}
            for li, name in ((1, "Wl1"), (2, "Wl2")):
                for r in range(NREL):
                    t = cpool.tile([H, H], BF16, tag=f"wl{li}{r}")
                    nc.sync.dma_start(t[:], p[name][r, :, :])
                    wl[(li, r)] = t
            wm = {}
            for name in ("Wm1", "Wm2"):
                t = cpool.tile([H, H], BF16, tag=name)
                nc.sync.dma_start(t[:], p[name][:, :])
                wm[name] = t
            wm3 = cpool.tile([H, C], BF16)
            nc.sync.dma_start(wm3[:], p["Wm3"][:, :])
            biases = {}
            for name in ("B1", "B2", "bm1", "bm2"):
                t = cpool.tile([H, 1], F32, tag=name)
                nc.sync.dma_start(t[:], p[name][:, :])
                biases[name] = t
            bm3 = cpool.tile([C, 1], F32)
            nc.sync.dma_start(bm3[:], p["bm3"][:, :])

            # ---- phase 0: h0 = relu(x @ W_in + b_in) for this core's shard
            ti = 0
            with tc.tile_pool(name="ph0", bufs=1) as hpool:
                xs = hpool.tile([IN + 1, SHARD], BF16)
                nc.sync.dma_start(xs[:], p["xTs"][:, :])
                for t in range(NBLK):
                    t0 = t * 128
                    tw = min(128, SHARD - t0)
                    ps = po.tile([128, H], F32, tag="mm")
                    nc.tensor.matmul(ps[:tw, :], lhsT=xs[:, t0:t0 + tw],
                                     rhs=w65[:], start=True, stop=True)
                    hb = wpool.tile([128, H], BF16, tag="h0out")
                    if ti % 2 == 0:
                        nc.scalar.activation(hb[:tw, :], ps[:tw, :],
                                             mybir.ActivationFunctionType.Relu)
                    else:
                        nc.vector.tensor_scalar_max(hb[:tw, :], ps[:tw, :], 0.0)
                    ti += 1
                    nc.sync.dma_start(h0_shard[t0:t0 + tw, :], hb[:tw, :])

            nc.gpsimd.collective_compute(
                "AllGather", mybir.AluOpType.bypass, replica_groups=rg,
                ins=[h0_shard[:, :]], outs=[h0_full[:, :]])

            # ---- conv layers
            def conv(layer, h_full):
                tables = [h_full[g * GRP:(g + 1) * GRP, :] for g in range(NGRP)]
                pooled = None
                if layer == 2:
                    pooled = pp.tile([H, G], F32, tag="pooled")
                ei = 0
                for sb in sbs:
                    agg = {}
                    for call in sb["calls"]:
                        g, kc, q0, d0, col0 = (call["g"], call["kc"],
                                               call["q0"], call["d0"],
                                               call["col0"])
                        nd = len(call["ohs"])
                        it = ipool.tile([128, KMAX * 8], I16, tag="it")
                        nc.sync.dma_start(it[:, :kc * 8],
                                          p["IDX"][:, col0:col0 + kc * 8])
                        st = dpool.tile([128, KMAX, H], BF16, tag="st")
                        nc.gpsimd.dma_gather(
                            st[:, :kc, :], tables[g], it[:, :kc * 8],
                            kc * 128, kc * 128, H, single_packet=False)
                        dl = mpool.tile([128, KMAX + SBS], BF16, tag="dl")
                        nc.scalar.dma_start(dl[:, :nd], p["DSL"][:, d0:d0 + nd])
                        dh = mpool.tile([128, KMAX + SBS], BF16, tag="dh")
                        nc.scalar.dma_start(dh[:, :nd], p["DSH"][:, d0:d0 + nd])
                        wc = mpool.tile([128, KMAX], F32, tag="wc")
                        nc.scalar.dma_start(wc[:, :kc], p["WC"][:, q0:q0 + kc])
                        # two bf16-native 256-wide compares per call build all
                        # one-hots (keep DVE light: its SBUF port contends
                        # with the gpsimd descriptor generator)
                        oh = opool.tile([128, KMAX + SBS, OW], BF16, tag="oh")
                        iview = iota_f[:, :].rearrange(
                            "p (o f) -> p o f", o=1).broadcast_to(
                                [128, nd, 256])
                        nc.vector.tensor_tensor(
                            out=oh[:, :nd, 0:256],
                            in0=dl[:, :nd].to_broadcast([128, nd, 256]),
                            in1=iview,
                            op=mybir.AluOpType.is_equal)
                        nc.vector.tensor_tensor(
                            out=oh[:, :nd, 256:OW],
                            in0=dh[:, :nd].to_broadcast([128, nd, 256]),
                            in1=iview,
                            op=mybir.AluOpType.is_equal)
                        # edge-weight scaling on the Scalar engine (own port)
                        sc = spool.tile([128, KMAX, H], BF16, tag="sc")
                        for i in range(kc):
                            nc.scalar.activation(
                                sc[:, i, :], st[:, i, :],
                                mybir.ActivationFunctionType.Copy,
                                scale=wc[:, i:i + 1])
                        for (ci, oi, b, first, last) in call["mms"]:
                            if b not in agg:
                                agg[b] = pa.tile([128, OW], F32,
                                                 name=f"agg{b % SBS}",
                                                 tag=f"agg{b % SBS}")
                            nc.tensor.matmul(agg[b][:], lhsT=sc[:, ci, :],
                                             rhs=oh[:, oi, :],
                                             start=first, stop=last)
                    # superblock epilogues
                    for b in sb["blocks"]:
                        rows = min(128, SHARD - b * 128)
                        ag4 = apool.tile([128, OW], BF16, tag=f"as{b % 2}")
                        if sb["tb"][b] == 0:
                            nc.vector.memset(ag4[:], 0.0)
                        else:
                            nc.scalar.activation(
                                ag4[:], agg[b][:],
                                mybir.ActivationFunctionType.Copy)
                        ei += 1
                        out2 = po.tile([128, 128], F32, tag="mm")
                        for r in range(NREL):
                            nc.tensor.matmul(
                                out2[:], lhsT=wl[(layer, r)][:],
                                rhs=ag4[:, r * 128:(r + 1) * 128],
                                start=(r == 0), stop=(r == NREL - 1))
                        if layer == 1:
                            t1 = wpool.tile([128, 128], BF16, tag="t1")
                            nc.scalar.activation(
                                t1[:], out2[:],
                                mybir.ActivationFunctionType.Relu,
                                bias=biases["B1"][:, :])
                            tb_ = pb.tile([128, 128], BF16, tag="tb")
                            nc.tensor.transpose(tb_[:], t1[:], id_bf[:])
                            t2 = wpool.tile([128, 128], BF16, tag="t2")
                            nc.scalar.activation(
                                t2[:], tb_[:],
                                mybir.ActivationFunctionType.Copy)
                            nc.sync.dma_start(
                                h1_shard[b * 128:b * 128 + rows, :],
                                t2[:rows, :])
                        else:
                            t1 = wpool.tile([128, 128], BF16, tag="t1")
                            nc.scalar.activation(
                                t1[:], out2[:],
                                mybir.ActivationFunctionType.Copy)
                            tb_ = pb.tile([128, 128], BF16, tag="tb")
                            nc.tensor.transpose(tb_[:], t1[:], id_bf[:])
                            t2 = wpool.tile([128, 128], BF16, tag="t2")
                            nc.scalar.activation(
                                t2[:], tb_[:],
                                mybir.ActivationFunctionType.Copy)
                            gm = wpool.tile([128, G], BF16, tag="gm")
                            nc.scalar.dma_start(
                                gm[:rows, :],
                                p["gmat"][b * 128:b * 128 + rows, :])
                            nc.tensor.matmul(pooled[:], lhsT=t2[:rows, :],
                                             rhs=gm[:rows, :],
                                             start=(b == 0),
                                             stop=(b == NBLK - 1))
                return pooled

            conv(1, h0_full)
            nc.gpsimd.collective_compute(
                "AllGather", mybir.AluOpType.bypass, replica_groups=rg,
                ins=[h1_shard[:, :]], outs=[h1_full[:, :]])
            pooled = conv(2, h1_full)

            # ---- pooled partial sums -> AllReduce -> head
            psb = wpool.tile([H, G], F32, tag="psb")
            nc.scalar.activation(psb[:], pooled[:],
                                 mybir.ActivationFunctionType.Copy)
            nc.sync.dma_start(pool_in[:, :], psb[:])
            nc.gpsimd.collective_compute(
                "AllReduce", mybir.AluOpType.add, replica_groups=rg,
                ins=[pool_in[:, :]], outs=[pool_out[:, :]])
            pool_f = wpool.tile([H, G], F32, tag="pool_f")
            nc.sync.dma_start(pool_f[:], pool_out[:, :])
            # + B2 (conv2 bias, linear through the mean), cast to bf16
            pool_b = wpool.tile([H, G], BF16, tag="pool_b")
            nc.vector.tensor_scalar_add(pool_b[:], pool_f[:], biases["B2"][:, :])

            z1p = po.tile([H, G], F32, tag="mm")
            nc.tensor.matmul(z1p[:], lhsT=wm["Wm1"][:], rhs=pool_b[:],
                             start=True, stop=True)
            z1 = wpool.tile([H, G], BF16, tag="z1")
            nc.scalar.activation(z1[:], z1p[:],
                                 mybir.ActivationFunctionType.Relu,
                                 bias=biases["bm1"][:, :])
            z2p = po.tile([H, G], F32, tag="mm")
            nc.tensor.matmul(z2p[:], lhsT=wm["Wm2"][:], rhs=z1[:],
                             start=True, stop=True)
            z2 = wpool.tile([H, G], BF16, tag="z2")
            nc.scalar.activation(z2[:], z2p[:],
                                 mybir.ActivationFunctionType.Relu,
                                 bias=biases["bm2"][:, :])
            z3p = po.tile([C, G], F32, tag="mm")
            nc.tensor.matmul(z3p[:], lhsT=wm3[:], rhs=z2[:],
                             start=True, stop=True)
            z3 = wpool.tile([C, G], F32, tag="z3")
            nc.vector.tensor_scalar_add(z3[:], z3p[:], bm3[:, :])
            nc.sync.dma_start(out_ext[:, :], z3[:])

    nc.compile()
    return nc


# ---------------------------------------------------------------------------
# entry point
# ---------------------------------------------------------------------------

_CACHE = {}


def kernel(x, src, dst, graph_ids, W_in, b_in, W1, b1, W2, b2,
           Wm1, bm1, Wm2, bm2, Wm3, bm3):
    x = np.asarray(x)
    key = (int(np.asarray(src).sum()) ^ int(np.asarray(dst).sum()),
           int(np.asarray(graph_ids).sum()))
    if key not in _CACHE:
        plan = _plan(src, dst, graph_ids)
        nc = _build(plan)
        _CACHE[key] = (plan, nc)
    plan, nc = _CACHE[key]

    xT = np.concatenate([np.asarray(x).T, np.ones((1, N), np.float32)], axis=0)
    w65 = np.concatenate([np.asarray(W_in), np.asarray(b_in)[None, :]], axis=0)

    def bf(a):
        return np.ascontiguousarray(np.asarray(a), dtype=np.float32).astype(bfloat16)

    def col(a):
        return np.ascontiguousarray(np.asarray(a, np.float32).reshape(-1, 1))

    xTb = bf(xT)
    in_maps = []
    for c in range(CORES):
        in_maps.append({
            "xTs": np.ascontiguousarray(xTb[:, c * SHARD:(c + 1) * SHARD]),
            "W65": bf(w65),
            "Wl1": bf(W1),
            "Wl2": bf(W2),
            "Wm1": bf(Wm1),
            "Wm2": bf(Wm2),
            "Wm3": bf(Wm3),
            "B1": col(np.asarray(b1, np.float32).sum(axis=0)),
            "B2": col(np.asarray(b2, np.float32).sum(axis=0)),
            "bm1": col(bm1),
            "bm2": col(bm2),
            "bm3": col(bm3),
            "IDX": np.ascontiguousarray(plan["IDX"][c]),
            "DSL": np.ascontiguousarray(plan["DSL"][c]),
            "DSH": np.ascontiguousarray(plan["DSH"][c]),
            "WC": np.ascontiguousarray(plan["WC"][c]),
            "gmat": np.ascontiguousarray(plan["gmat"][c]),
        })

    res = bass_utils.run_bass_kernel_spmd(nc, in_maps, list(range(CORES)))
    global LAST_EXEC_NS
    LAST_EXEC_NS = res.exec_time_ns
    out = np.asarray(res.results[0]["out"], np.float32)  # [C, G]
    return np.ascontiguousarray(out.T)                   # [G, C]


LAST_EXEC_NS = None


if __name__ == "__main__":
    import reference
    import jax
    with jax.default_device(jax.devices("cpu")[0]):
        inp = {k: np.asarray(v) for k, v in reference.setup_inputs().items()}
        exp = np.asarray(reference.reference(**{k: v for k, v in inp.items()}))
    act = kernel(**inp)
    rel = np.linalg.norm(act - exp) / np.linalg.norm(exp)
    print("Relative error:", rel)



# revision 5
# speedup vs baseline: 1.4086x; 1.4086x over previous
"""RGCN (2x hetero GraphConv + mean-pool + MLP) on 8 TRN2 NeuronCores — v3.

Key structure (vs v2 baseline):
- Layer 2 + mean-pooling are algebraically fused into a dense matmul: with no
  relu after conv2 and pooling linear, pooled[g] only needs
  M[n,(r,g)] = sum_{e in r: src=n, gid(dst)=g} w_e / c_g (host-built graph
  metadata). Per dst block: pool += h1_block^T @ M_block. This removes the
  second gather/scatter pass and the h1 AllGather entirely.
- Layer 1 keeps the dst-sharded gather/one-hot-scatter design but with
  relation-pure 128-edge chunks (cells keyed (block, relation, src-group)) so
  one-hots are 128 wide (4x less PE/DVE work than the 512-wide merged form).
- dma_gather calls round-robin across 4 SWDGE queues; each queue maps to a
  different GpSimd Q7 cpu pair, so descriptor generation (the old wall at
  ~9.3 ns/row) runs 4-wide (~2.1 ns/row).
- Per-edge normalization w_e = rsqrt(deg_out_r[src])*rsqrt(deg_in_r[dst]) is a
  tensor_tensor mult of the gathered rows by a broadcast weight column;
  split between DVE and GpSimd to balance engine load.

SPMD: identical instruction stream on all 8 cores; per-core variation lives in
IDX/DSLWC/M tensors. Chunk counts per cell are the max over cores.
"""

import numpy as np
from ml_dtypes import bfloat16

import concourse.bass as bass
import concourse.bacc as bacc
import concourse.mybir as mybir
import concourse.tile as tile
from concourse import bass_utils
from concourse.masks import make_identity

F32 = mybir.dt.float32
BF16 = mybir.dt.bfloat16
I16 = mybir.dt.int16
I32 = mybir.dt.int32

# problem constants (hardcoded per spec)
N, E, NREL, G, IN, H, C = 100000, 400000, 4, 64, 64, 128, 2
CORES = 8
SHARD = N // CORES            # 12500
NBLK = (SHARD + 127) // 128   # 98
GRP = 25000                   # src rows per gather group (int16 idx limit)
NGRP = N // GRP               # 4
SBS = 4                       # dst blocks per superblock (PSUM banks)
KMAX = 32                     # max 128-edge chunks per gather call
MC = NREL * G                 # pooling matrix columns: 256


# ---------------------------------------------------------------------------
# host-side planning: pure graph-structure metadata (indices, degrees, layout)
# ---------------------------------------------------------------------------

def _plan(src, dst, graph_ids):
    src = np.asarray(src).astype(np.int64)
    dst = np.asarray(dst).astype(np.int64)
    gid = np.asarray(graph_ids).astype(np.int64)

    # folded normalization: w_e = rsqrt(deg_in[dst]) * rsqrt(deg_out[src])
    w_all = np.empty((NREL, E), np.float32)
    for r in range(NREL):
        do = np.maximum(np.bincount(src[r], minlength=N), 1.0)
        di = np.maximum(np.bincount(dst[r], minlength=N), 1.0)
        w_all[r] = (1.0 / np.sqrt(do[src[r]]) / np.sqrt(di[dst[r]])).astype(np.float32)

    # cells keyed (block, relation, group); count per core
    NC_CELL = NBLK * NREL * NGRP
    cnt = np.zeros((CORES, NC_CELL), np.int64)
    cell_data = [[None] * NC_CELL for _ in range(CORES)]
    for c in range(CORES):
        for r in range(NREL):
            local = dst[r] - c * SHARD
            m = (local >= 0) & (local < SHARD)
            es = np.nonzero(m)[0]
            loc = local[es]
            b_arr = loc // 128
            g_arr = src[r][es] // GRP
            key = (b_arr * NREL + r) * NGRP + g_arr
            order = np.argsort(key, kind="stable")
            es, loc, key = es[order], loc[order], key[order]
            bounds = np.searchsorted(key, np.arange(NC_CELL + 1))
            si_all = (src[r][es] % GRP).astype(np.int16)
            dl_all = (loc % 128).astype(np.int16)
            wv_all = w_all[r][es].astype(np.float32)
            for b in range(NBLK):
                for g in range(NGRP):
                    cell = (b * NREL + r) * NGRP + g
                    s0, s1 = bounds[cell], bounds[cell + 1]
                    if s0 == s1:
                        continue
                    cell_data[c][cell] = (si_all[s0:s1], dl_all[s0:s1],
                                          wv_all[s0:s1])
                    cnt[c, cell] = s1 - s0

    nch = -(-cnt.max(axis=0) // 128)          # [NC_CELL] chunks per cell

    # call layout: per superblock, per group, chunks from cells
    # (b in sb, r in 0..3); split into calls of <= KMAX chunks.
    sbs = []
    q = 0       # global chunk counter (DSL/WC columns)
    col = 0     # IDX int16 column counter
    for s0 in range(0, NBLK, SBS):
        blocks = list(range(s0, min(s0 + SBS, NBLK)))
        calls = []
        # first/last bookkeeping per (b, r) across the sb
        br_chunks = {}
        for g in range(NGRP):
            descs = []
            for b in blocks:
                for r in range(NREL):
                    cell = (b * NREL + r) * NGRP + g
                    for k in range(int(nch[cell])):
                        descs.append((b, r, g, k, cell))
            for i0 in range(0, len(descs), KMAX):
                part = descs[i0:i0 + KMAX]
                call = dict(g=g, kc=len(part), q0=q, col0=col, part=part)
                calls.append(call)
                for i, (b, r, g_, k, cell) in enumerate(part):
                    br_chunks.setdefault((b, r), []).append(
                        (len(calls) - 1, i))
                q += len(part)
                col += len(part) * 8
        # flags: stop on the last matmul targeting block b within the sb
        b_last = {}
        for ci, call in enumerate(calls):
            for i, (b, r, g_, k, cell) in enumerate(call["part"]):
                b_last[b] = (ci, i)
        for ci, call in enumerate(calls):
            mms = []
            for i, (b, r, g_, k, cell) in enumerate(call["part"]):
                last = b_last.get(b) == (ci, i)
                mms.append((i, b, r, False, last))
            call["mms"] = mms
        empty = [(b, r) for b in blocks for r in range(NREL)
                 if (b, r) not in br_chunks]
        sbs.append(dict(blocks=blocks, calls=calls, empty=empty))
    NCH = q
    NIDXCOL = col

    # chunk data arrays
    IDX = np.zeros((CORES, 16, NIDXCOL), np.int16)
    DSL = np.full((CORES, 128, NCH), -1.0, np.float32)
    WC = np.zeros((CORES, 128, NCH), np.float32)
    for c in range(CORES):
        for sb in sbs:
            for call in sb["calls"]:
                for i, (b, r, g, k, cell) in enumerate(call["part"]):
                    data = cell_data[c][cell]
                    if data is None:
                        continue
                    si, dl, wv = data
                    v0, v1 = k * 128, min((k + 1) * 128, len(si))
                    take = v1 - v0
                    if take <= 0:
                        continue
                    qq = call["q0"] + i
                    nn = i * 128 + np.arange(take)
                    IDX[c, nn % 16, call["col0"] + nn // 16] = si[v0:v1]
                    DSL[c, :take, qq] = dl[v0:v1]
                    WC[c, :take, qq] = wv[v0:v1]

    # pooling matrix M[n, r*G + g] = sum_{e in r: src=n, gid(dst)=g} w_e / c_g
    cnt_g = np.maximum(np.bincount(gid, minlength=G), 1.0)
    M = np.zeros((N, MC), np.float32)
    for r in range(NREL):
        gd = gid[dst[r]]
        np.add.at(M, (src[r], r * G + gd), w_all[r] / cnt_g[gd])
    Mpad = np.zeros((CORES, NBLK * 128, MC), np.float32)
    for c in range(CORES):
        Mpad[c, :SHARD] = M[c * SHARD:(c + 1) * SHARD]

    return dict(sbs=sbs, NCH=NCH, NIDXCOL=NIDXCOL,
                IDX=np.tile(IDX, (1, 8, 1)),
                DSL=DSL.astype(bfloat16), WC=WC.astype(bfloat16),
                M=Mpad.astype(bfloat16))


# ---------------------------------------------------------------------------
# device program
# ---------------------------------------------------------------------------

def _build(plan):
    sbs = plan["sbs"]
    NCH = plan["NCH"]
    NIDXCOL = plan["NIDXCOL"]

    nc = bacc.Bacc(None, target_bir_lowering=False, num_devices=CORES,
                   num_swdge_queues=4)

    p = {}
    p["xTs"] = nc.declare_dram_parameter("xTs", [IN + 1, SHARD], BF16, isOutput=False)
    p["W65"] = nc.declare_dram_parameter("W65", [IN + 1, H], BF16, isOutput=False)
    p["Wl1"] = nc.declare_dram_parameter("Wl1", [NREL, H, H], BF16, isOutput=False)
    p["Wl2"] = nc.declare_dram_parameter("Wl2", [NREL, H, H], BF16, isOutput=False)
    p["Wm1"] = nc.declare_dram_parameter("Wm1", [H, H], BF16, isOutput=False)
    p["Wm2"] = nc.declare_dram_parameter("Wm2", [H, H], BF16, isOutput=False)
    p["Wm3"] = nc.declare_dram_parameter("Wm3", [H, C], BF16, isOutput=False)
    p["B1"] = nc.declare_dram_parameter("B1", [H, 1], F32, isOutput=False)
    p["B2"] = nc.declare_dram_parameter("B2", [H, 1], F32, isOutput=False)
    p["bm1"] = nc.declare_dram_parameter("bm1", [H, 1], F32, isOutput=False)
    p["bm2"] = nc.declare_dram_parameter("bm2", [H, 1], F32, isOutput=False)
    p["bm3"] = nc.declare_dram_parameter("bm3", [C, 1], F32, isOutput=False)
    p["IDX"] = nc.declare_dram_parameter("IDX", [128, NIDXCOL], I16, isOutput=False)
    p["DSL"] = nc.declare_dram_parameter("DSL", [128, NCH], BF16, isOutput=False)
    p["WC"] = nc.declare_dram_parameter("WC", [128, NCH], BF16, isOutput=False)
    p["M"] = nc.declare_dram_parameter("M", [NBLK * 128, MC], BF16, isOutput=False)
    out_ext = nc.declare_dram_parameter("out", [C, G], F32, isOutput=True)

    h0_shard = nc.dram_tensor("h0_shard", [SHARD, H], BF16)
    h0_full = nc.dram_tensor("h0_full", [N, H], BF16, addr_space="Shared")
    pool_in = nc.dram_tensor("pool_in", [H, MC], F32)
    pool_out = nc.dram_tensor("pool_out", [H, MC], F32, addr_space="Shared")

    rg = [list(range(CORES))]

    with tile.TileContext(nc) as tc:
        with (
            tc.tile_pool(name="const", bufs=1) as cpool,
            tc.tile_pool(name="idx", bufs=4) as ipool,
            tc.tile_pool(name="meta", bufs=4) as mpool,
            tc.tile_pool(name="gath", bufs=4) as dpool,
            tc.tile_pool(name="scal", bufs=3) as spool,
            tc.tile_pool(name="oneh", bufs=3) as opool,
            tc.tile_pool(name="aggs", bufs=2) as apool,
            tc.tile_pool(name="work", bufs=4) as wpool,
            tc.tile_pool(name="pa", bufs=1, space="PSUM") as pa,
            tc.tile_pool(name="po", bufs=2, space="PSUM") as po,
            tc.tile_pool(name="pb", bufs=1, space="PSUM") as pb,
            tc.tile_pool(name="pp", bufs=1, space="PSUM") as pp,
        ):
            # ---- constants
            id_f32 = cpool.tile([128, 128], F32)
            make_identity(nc, id_f32[:])
            id_bf = cpool.tile([128, 128], BF16)
            nc.vector.tensor_copy(id_bf[:], id_f32[:])

            zl = cpool.tile([128, 128], BF16, tag="zl")
            nc.vector.memset(zl[:], 0.0)
            zr = cpool.tile([128, NREL * 128], BF16, tag="zr")
            nc.vector.memset(zr[:], 0.0)

            iota_i = cpool.tile([128, 128], I32)
            nc.gpsimd.iota(iota_i[:], pattern=[[1, 128]], base=0,
                           channel_multiplier=0)
            iota_f = cpool.tile([128, 128], BF16)
            nc.vector.tensor_copy(iota_f[:], iota_i[:])

            w65 = cpool.tile([IN + 1, H], BF16)
            nc.sync.dma_start(w65[:], p["W65"][:, :])
            wl = {}
            for li, name in ((1, "Wl1"), (2, "Wl2")):
                for r in range(NREL):
                    t = cpool.tile([H, H], BF16, tag=f"wl{li}{r}")
                    nc.sync.dma_start(t[:], p[name][r, :, :])
                    wl[(li, r)] = t
            wm = {}
            for name in ("Wm1", "Wm2"):
                t = cpool.tile([H, H], BF16, tag=name)
                nc.sync.dma_start(t[:], p[name][:, :])
                wm[name] = t
            wm3 = cpool.tile([H, C], BF16)
            nc.sync.dma_start(wm3[:], p["Wm3"][:, :])
            biases = {}
            for name in ("B1", "B2", "bm1", "bm2"):
                t = cpool.tile([H, 1], F32, tag=name)
                nc.sync.dma_start(t[:], p[name][:, :])
                biases[name] = t
            bm3 = cpool.tile([C, 1], F32)
            nc.sync.dma_start(bm3[:], p["bm3"][:, :])

            # ---- phase 0: h0 = relu(x @ W_in + b_in) for this core's shard
            with tc.tile_pool(name="ph0", bufs=1) as hpool:
                xs = hpool.tile([IN + 1, SHARD], BF16)
                nc.sync.dma_start(xs[:], p["xTs"][:, :])
                for t in range(NBLK):
                    t0 = t * 128
                    tw = min(128, SHARD - t0)
                    ps = po.tile([128, H], F32, tag="mm")
                    nc.tensor.matmul(ps[:tw, :], lhsT=xs[:, t0:t0 + tw],
                                     rhs=w65[:], start=True, stop=True)
                    hb = wpool.tile([128, H], BF16, tag="h0out")
                    if t % 2 == 0:
                        nc.scalar.activation(hb[:tw, :], ps[:tw, :],
                                             mybir.ActivationFunctionType.Relu)
                    else:
                        nc.vector.tensor_scalar_max(hb[:tw, :], ps[:tw, :], 0.0)
                    nc.sync.dma_start(h0_shard[t0:t0 + tw, :], hb[:tw, :])

            nc.gpsimd.collective_compute(
                "AllGather", mybir.AluOpType.bypass, replica_groups=rg,
                ins=[h0_shard[:, :]], outs=[h0_full[:, :]])

            tables = [h0_full[g * GRP:(g + 1) * GRP, :] for g in range(NGRP)]
            pooled = pp.tile([H, MC], F32, tag="pooled")

            # ---- conv1 + fused pooling
            qi = 0  # running call index for queue round-robin / engine split
            for sb in sbs:
                agg = {}
                for call in sb["calls"]:
                    g, kc, q0, col0 = call["g"], call["kc"], call["q0"], call["col0"]
                    it = ipool.tile([128, KMAX * 8], I16, tag="it")
                    nc.sync.dma_start(it[:, :kc * 8],
                                      p["IDX"][:, col0:col0 + kc * 8])
                    st = dpool.tile([128, KMAX, H], BF16, tag="st")
                    nc.gpsimd.dma_gather(
                        st[:, :kc, :], tables[g], it[:, :kc * 8],
                        kc * 128, kc * 128, H, single_packet=False,
                        queue_num=qi % 4)
                    dslt = mpool.tile([128, KMAX], BF16, tag="dsl")
                    nc.scalar.dma_start(dslt[:, :kc], p["DSL"][:, q0:q0 + kc])
                    wct = mpool.tile([128, KMAX], BF16, tag="wc")
                    nc.scalar.dma_start(wct[:, :kc], p["WC"][:, q0:q0 + kc])
                    # one-hot build and edge-weight scale, both on DVE
                    oh = opool.tile([128, KMAX, 128], BF16, tag="oh")
                    iview = iota_f[:, :].rearrange("p (o f) -> p o f", o=1)\
                        .broadcast_to([128, kc, 128])
                    nc.vector.tensor_tensor(
                        out=oh[:, :kc, :],
                        in0=dslt[:, :kc].to_broadcast([128, kc, 128]),
                        in1=iview, op=mybir.AluOpType.is_equal)
                    sc = spool.tile([128, KMAX, H], BF16, tag="sc")
                    nc.vector.tensor_tensor(
                        out=sc[:, :kc, :], in0=st[:, :kc, :],
                        in1=wct[:, :kc].to_broadcast([128, kc, 128]),
                        op=mybir.AluOpType.mult)
                    for (i, b, r, first, last) in call["mms"]:
                        if b not in agg:
                            agg[b] = pa.tile([128, NREL * 128], F32,
                                             name=f"agg{b % SBS}",
                                             tag=f"agg{b % SBS}")
                            # start=True zeroes the whole PSUM bank, so do it
                            # once with a full-width zero matmul; the real
                            # slice matmuls then all accumulate.
                            nc.tensor.matmul(agg[b][:], lhsT=zl[:], rhs=zr[:],
                                             start=True, stop=False)
                        nc.tensor.matmul(agg[b][:, r * 128:(r + 1) * 128],
                                         lhsT=sc[:, i, :], rhs=oh[:, i, :],
                                         start=False, stop=last)
                    qi += 1
                # superblock epilogue
                for b in sb["blocks"]:
                    rows = min(128, SHARD - b * 128)
                    ag4 = apool.tile([128, NREL * 128], BF16, tag=f"as{b % 2}")
                    if b in agg:
                        nc.scalar.activation(
                            ag4[:], agg[b][:],
                            mybir.ActivationFunctionType.Copy)
                    else:
                        nc.vector.memset(ag4[:], 0.0)
                    for (eb, er) in sb["empty"]:
                        if eb == b and (b in agg):
                            nc.vector.memset(
                                ag4[:, er * 128:(er + 1) * 128], 0.0)
                    out2 = po.tile([128, 128], F32, tag="mm")
                    for r in range(NREL):
                        nc.tensor.matmul(
                            out2[:], lhsT=wl[(1, r)][:],
                            rhs=ag4[:, r * 128:(r + 1) * 128],
                            start=(r == 0), stop=(r == NREL - 1))
                    t1 = wpool.tile([128, 128], BF16, tag="t1")
                    nc.scalar.activation(
                        t1[:], out2[:],
                        mybir.ActivationFunctionType.Relu,
                        bias=biases["B1"][:, :])
                    tb_ = pb.tile([128, 128], BF16, tag="tb")
                    nc.tensor.transpose(tb_[:], t1[:], id_bf[:])
                    t2 = wpool.tile([128, 128], BF16, tag="t2")
                    nc.scalar.activation(
                        t2[:], tb_[:],
                        mybir.ActivationFunctionType.Copy)
                    mb = wpool.tile([128, MC], BF16, tag="mb")
                    nc.scalar.dma_start(
                        mb[:rows, :], p["M"][b * 128:b * 128 + rows, :])
                    nc.tensor.matmul(pooled[:], lhsT=t2[:rows, :],
                                     rhs=mb[:rows, :],
                                     start=(b == 0), stop=(b == NBLK - 1))

            # ---- pooled partial sums -> AllReduce -> W2 contraction -> head
            psb = wpool.tile([H, MC], F32, tag="psb")
            nc.scalar.activation(psb[:], pooled[:],
                                 mybir.ActivationFunctionType.Copy)
            nc.sync.dma_start(pool_in[:, :], psb[:])
            nc.gpsimd.collective_compute(
                "AllReduce", mybir.AluOpType.add, replica_groups=rg,
                ins=[pool_in[:, :]], outs=[pool_out[:, :]])
            pool_f32 = wpool.tile([H, MC], F32, tag="pool_f32")
            nc.sync.dma_start(pool_f32[:], pool_out[:, :])
            pool_f = wpool.tile([H, MC], BF16, tag="pool_f")
            nc.vector.tensor_copy(pool_f[:], pool_f32[:])

            z0p = po.tile([H, G], F32, tag="mm")
            for r in range(NREL):
                nc.tensor.matmul(z0p[:], lhsT=wl[(2, r)][:],
                                 rhs=pool_f[:, r * G:(r + 1) * G],
                                 start=(r == 0), stop=(r == NREL - 1))
            # + B2 (conv2 bias, linear through the mean), cast to bf16
            zb = wpool.tile([H, G], BF16, tag="zb")
            nc.vector.tensor_scalar_add(zb[:], z0p[:], biases["B2"][:, :])

            z1p = po.tile([H, G], F32, tag="mm")
            nc.tensor.matmul(z1p[:], lhsT=wm["Wm1"][:], rhs=zb[:],
                             start=True, stop=True)
            z1 = wpool.tile([H, G], BF16, tag="z1")
            nc.scalar.activation(z1[:], z1p[:],
                                 mybir.ActivationFunctionType.Relu,
                                 bias=biases["bm1"][:, :])
            z2p = po.tile([H, G], F32, tag="mm")
            nc.tensor.matmul(z2p[:], lhsT=wm["Wm2"][:], rhs=z1[:],
                             start=True, stop=True)
            z2 = wpool.tile([H, G], BF16, tag="z2")
            nc.scalar.activation(z2[:], z2p[:],
                                 mybir.ActivationFunctionType.Relu,
                                 bias=biases["bm2"][:, :])
            z3p = po.tile([C, G], F32, tag="mm")
            nc.tensor.matmul(z3p[:], lhsT=wm3[:], rhs=z2[:],
                             start=True, stop=True)
            z3 = wpool.tile([C, G], F32, tag="z3")
            nc.vector.tensor_scalar_add(z3[:], z3p[:], bm3[:, :])
            nc.sync.dma_start(out_ext[:, :], z3[:])

    nc.compile()
    return nc


# ---------------------------------------------------------------------------
# entry point
# ---------------------------------------------------------------------------

_CACHE = {}


def kernel(x, src, dst, graph_ids, W_in, b_in, W1, b1, W2, b2,
           Wm1, bm1, Wm2, bm2, Wm3, bm3):
    x = np.asarray(x)
    key = (int(np.asarray(src).sum()) ^ int(np.asarray(dst).sum()),
           int(np.asarray(graph_ids).sum()))
    if key not in _CACHE:
        plan = _plan(src, dst, graph_ids)
        nc = _build(plan)
        _CACHE[key] = (plan, nc)
    plan, nc = _CACHE[key]

    xT = np.concatenate([np.asarray(x).T, np.ones((1, N), np.float32)], axis=0)
    w65 = np.concatenate([np.asarray(W_in), np.asarray(b_in)[None, :]], axis=0)

    def bf(a):
        return np.ascontiguousarray(np.asarray(a), dtype=np.float32).astype(bfloat16)

    def col(a):
        return np.ascontiguousarray(np.asarray(a, np.float32).reshape(-1, 1))

    xTb = bf(xT)
    in_maps = []
    for c in range(CORES):
        in_maps.append({
            "xTs": np.ascontiguousarray(xTb[:, c * SHARD:(c + 1) * SHARD]),
            "W65": bf(w65),
            "Wl1": bf(W1),
            "Wl2": bf(W2),
            "Wm1": bf(Wm1),
            "Wm2": bf(Wm2),
            "Wm3": bf(Wm3),
            "B1": col(np.asarray(b1, np.float32).sum(axis=0)),
            "B2": col(np.asarray(b2, np.float32).sum(axis=0)),
            "bm1": col(bm1),
            "bm2": col(bm2),
            "bm3": col(bm3),
            "IDX": np.ascontiguousarray(plan["IDX"][c]),
            "DSL": np.ascontiguousarray(plan["DSL"][c]),
            "WC": np.ascontiguousarray(plan["WC"][c]),
            "M": np.ascontiguousarray(plan["M"][c]),
        })

    res = bass_utils.run_bass_kernel_spmd(nc, in_maps, list(range(CORES)))
    global LAST_EXEC_NS
    LAST_EXEC_NS = res.exec_time_ns
    out = np.asarray(res.results[0]["out"], np.float32)  # [C, G]
    return np.ascontiguousarray(out.T)                   # [G, C]


LAST_EXEC_NS = None


if __name__ == "__main__":
    import reference
    import jax
    with jax.default_device(jax.devices("cpu")[0]):
        inp = {k: np.asarray(v) for k, v in reference.setup_inputs().items()}
        exp = np.asarray(reference.reference(**{k: v for k, v in inp.items()}))
    act = kernel(**inp)
    rel = np.linalg.norm(act - exp) / np.linalg.norm(exp)
    print("Relative error:", rel)


# revision 8
# speedup vs baseline: 1.4659x; 1.0407x over previous
"""RGCN (2x hetero GraphConv + mean-pool + MLP) on 8 TRN2 NeuronCores — v3.

Key structure (vs v2 baseline):
- Layer 2 + mean-pooling are algebraically fused into a dense matmul: with no
  relu after conv2 and pooling linear, pooled[g] only needs
  M[n,(r,g)] = sum_{e in r: src=n, gid(dst)=g} w_e / c_g (host-built graph
  metadata). Per dst block: pool += h1_block^T @ M_block. This removes the
  second gather/scatter pass and the h1 AllGather entirely.
- Layer 1 keeps the dst-sharded gather/one-hot-scatter design but with
  relation-pure 128-edge chunks (cells keyed (block, relation, src-group)) so
  one-hots are 128 wide (4x less PE/DVE work than the 512-wide merged form).
- dma_gather calls round-robin across 4 SWDGE queues; each queue maps to a
  different GpSimd Q7 cpu pair, so descriptor generation (the old wall at
  ~9.3 ns/row) runs 4-wide (~2.1 ns/row).
- Per-edge normalization w_e = rsqrt(deg_out_r[src])*rsqrt(deg_in_r[dst]) is a
  tensor_tensor mult of the gathered rows by a broadcast weight column;
  split between DVE and GpSimd to balance engine load.

SPMD: identical instruction stream on all 8 cores; per-core variation lives in
IDX/DSLWC/M tensors. Chunk counts per cell are the max over cores.
"""

import numpy as np
from ml_dtypes import bfloat16

import concourse.bass as bass
import concourse.bacc as bacc
import concourse.mybir as mybir
import concourse.tile as tile
from concourse import bass_utils
from concourse.masks import make_identity

F32 = mybir.dt.float32
BF16 = mybir.dt.bfloat16
I16 = mybir.dt.int16
I32 = mybir.dt.int32

# problem constants (hardcoded per spec)
N, E, NREL, G, IN, H, C = 100000, 400000, 4, 64, 64, 128, 2
CORES = 8
SHARD = N // CORES            # 12500
NBLK = (SHARD + 127) // 128   # 98
GRP = 25000                   # src rows per gather group (int16 idx limit)
NGRP = N // GRP               # 4
SBS = 4                       # dst blocks per superblock (PSUM banks)
KMAX = 32                     # max 128-edge chunks per gather call
MC = NREL * G                 # pooling matrix columns: 256


# ---------------------------------------------------------------------------
# host-side planning: pure graph-structure metadata (indices, degrees, layout)
# ---------------------------------------------------------------------------

def _plan(src, dst, graph_ids):
    src = np.asarray(src).astype(np.int64)
    dst = np.asarray(dst).astype(np.int64)
    gid = np.asarray(graph_ids).astype(np.int64)

    # folded normalization: w_e = rsqrt(deg_in[dst]) * rsqrt(deg_out[src])
    w_all = np.empty((NREL, E), np.float32)
    for r in range(NREL):
        do = np.maximum(np.bincount(src[r], minlength=N), 1.0)
        di = np.maximum(np.bincount(dst[r], minlength=N), 1.0)
        w_all[r] = (1.0 / np.sqrt(do[src[r]]) / np.sqrt(di[dst[r]])).astype(np.float32)

    # cells keyed (block, relation, group); count per core
    NC_CELL = NBLK * NREL * NGRP
    cnt = np.zeros((CORES, NC_CELL), np.int64)
    cell_data = [[None] * NC_CELL for _ in range(CORES)]
    for c in range(CORES):
        for r in range(NREL):
            local = dst[r] - c * SHARD
            m = (local >= 0) & (local < SHARD)
            es = np.nonzero(m)[0]
            loc = local[es]
            b_arr = loc // 128
            g_arr = src[r][es] // GRP
            key = (b_arr * NREL + r) * NGRP + g_arr
            order = np.argsort(key, kind="stable")
            es, loc, key = es[order], loc[order], key[order]
            bounds = np.searchsorted(key, np.arange(NC_CELL + 1))
            si_all = (src[r][es] % GRP).astype(np.int16)
            dl_all = (loc % 128).astype(np.int16)
            wv_all = w_all[r][es].astype(np.float32)
            for b in range(NBLK):
                for g in range(NGRP):
                    cell = (b * NREL + r) * NGRP + g
                    s0, s1 = bounds[cell], bounds[cell + 1]
                    if s0 == s1:
                        continue
                    cell_data[c][cell] = (si_all[s0:s1], dl_all[s0:s1],
                                          wv_all[s0:s1])
                    cnt[c, cell] = s1 - s0

    nch = -(-cnt.max(axis=0) // 128)          # [NC_CELL] chunks per cell

    # call layout: per superblock, per group, chunks from cells
    # (b in sb, r in 0..3); split into calls of <= KMAX chunks.
    sbs = []
    q = 0       # global chunk counter (DSL/WC columns)
    col = 0     # IDX int16 column counter
    for s0 in range(0, NBLK, SBS):
        blocks = list(range(s0, min(s0 + SBS, NBLK)))
        calls = []
        # first/last bookkeeping per (b, r) across the sb
        br_chunks = {}
        for g in range(NGRP):
            descs = []
            tail = []
            for b in blocks:
                for r in range(NREL):
                    cell = (b * NREL + r) * NGRP + g
                    for k in range(int(nch[cell])):
                        if k == 0:
                            descs.append((b, r, g, k, cell))
                        else:
                            tail.append((b, r, g, k, cell))
            # overflow chunks last (likely empty on most cores -> their
            # trailing -1 idxs are trimmed by the Q7 gather at runtime);
            # order by descending mean count so emptier chunks go later
            tail.sort(key=lambda d: -int(cnt[:, d[4]].sum()))
            descs += tail
            for i0 in range(0, len(descs), KMAX):
                part = descs[i0:i0 + KMAX]
                call = dict(g=g, kc=len(part), q0=q, col0=col, part=part)
                calls.append(call)
                for i, (b, r, g_, k, cell) in enumerate(part):
                    br_chunks.setdefault((b, r), []).append(
                        (len(calls) - 1, i))
                q += len(part)
                col += len(part) * 8
        # flags: stop on the last matmul targeting block b within the sb
        b_last = {}
        for ci, call in enumerate(calls):
            for i, (b, r, g_, k, cell) in enumerate(call["part"]):
                b_last[b] = (ci, i)
        for ci, call in enumerate(calls):
            mms = []
            for i, (b, r, g_, k, cell) in enumerate(call["part"]):
                last = b_last.get(b) == (ci, i)
                mms.append((i, b, r, False, last))
            call["mms"] = mms
        empty = [(b, r) for b in blocks for r in range(NREL)
                 if (b, r) not in br_chunks]
        sbs.append(dict(blocks=blocks, calls=calls, empty=empty))
    NCH = q
    NIDXCOL = col

    # chunk data arrays
    IDX = np.zeros((CORES, 16, NIDXCOL), np.int16)
    DSL = np.full((CORES, 128, NCH), -1.0, np.float32)
    WC = np.zeros((CORES, 128, NCH), np.float32)
    for c in range(CORES):
        for sb in sbs:
            for call in sb["calls"]:
                kc = call["kc"]
                last_real = -1  # last idx slot (within call) holding a real edge
                for i, (b, r, g, k, cell) in enumerate(call["part"]):
                    data = cell_data[c][cell]
                    if data is None:
                        continue
                    si, dl, wv = data
                    v0, v1 = k * 128, min((k + 1) * 128, len(si))
                    take = v1 - v0
                    if take <= 0:
                        continue
                    qq = call["q0"] + i
                    nn = i * 128 + np.arange(take)
                    IDX[c, nn % 16, call["col0"] + nn // 16] = si[v0:v1]
                    DSL[c, :take, qq] = dl[v0:v1]
                    WC[c, :take, qq] = wv[v0:v1]
                    last_real = max(last_real, i * 128 + take - 1)
                # (trailing -1 trim disabled: caused device crash)
                del last_real

    # pooling matrix M[n, r*G + g] = sum_{e in r: src=n, gid(dst)=g} w_e / c_g
    cnt_g = np.maximum(np.bincount(gid, minlength=G), 1.0)
    M = np.zeros((N, MC), np.float32)
    for r in range(NREL):
        gd = gid[dst[r]]
        np.add.at(M, (src[r], r * G + gd), w_all[r] / cnt_g[gd])
    Mpad = np.zeros((CORES, NBLK * 128, MC), np.float32)
    for c in range(CORES):
        Mpad[c, :SHARD] = M[c * SHARD:(c + 1) * SHARD]

    return dict(sbs=sbs, NCH=NCH, NIDXCOL=NIDXCOL,
                IDX=np.tile(IDX, (1, 8, 1)),
                DSL=DSL.astype(bfloat16), WC=WC.astype(bfloat16),
                M=Mpad.astype(bfloat16))


# ---------------------------------------------------------------------------
# device program
# ---------------------------------------------------------------------------

def _build(plan):
    sbs = plan["sbs"]
    NCH = plan["NCH"]
    NIDXCOL = plan["NIDXCOL"]

    nc = bacc.Bacc(None, target_bir_lowering=False, num_devices=CORES,
                   num_swdge_queues=4)

    p = {}
    p["xTs"] = nc.declare_dram_parameter("xTs", [IN + 1, SHARD], BF16, isOutput=False)
    p["W65"] = nc.declare_dram_parameter("W65", [IN + 1, H], BF16, isOutput=False)
    p["Wl1"] = nc.declare_dram_parameter("Wl1", [NREL, H, H], BF16, isOutput=False)
    p["Wl2"] = nc.declare_dram_parameter("Wl2", [NREL, H, H], BF16, isOutput=False)
    p["Wm1"] = nc.declare_dram_parameter("Wm1", [H, H], BF16, isOutput=False)
    p["Wm2"] = nc.declare_dram_parameter("Wm2", [H, H], BF16, isOutput=False)
    p["Wm3"] = nc.declare_dram_parameter("Wm3", [H, C], BF16, isOutput=False)
    p["B1"] = nc.declare_dram_parameter("B1", [H, 1], F32, isOutput=False)
    p["B2"] = nc.declare_dram_parameter("B2", [H, 1], F32, isOutput=False)
    p["bm1"] = nc.declare_dram_parameter("bm1", [H, 1], F32, isOutput=False)
    p["bm2"] = nc.declare_dram_parameter("bm2", [H, 1], F32, isOutput=False)
    p["bm3"] = nc.declare_dram_parameter("bm3", [C, 1], F32, isOutput=False)
    p["IDX"] = nc.declare_dram_parameter("IDX", [128, NIDXCOL], I16, isOutput=False)
    p["DSL"] = nc.declare_dram_parameter("DSL", [128, NCH], BF16, isOutput=False)
    p["WC"] = nc.declare_dram_parameter("WC", [128, NCH], BF16, isOutput=False)
    p["M"] = nc.declare_dram_parameter("M", [NBLK * 128, MC], BF16, isOutput=False)
    out_ext = nc.declare_dram_parameter("out", [C, G], F32, isOutput=True)

    h0_shard = nc.dram_tensor("h0_shard", [SHARD, H], BF16)
    h0_full = nc.dram_tensor("h0_full", [N, H], BF16, addr_space="Shared")
    pool_in = nc.dram_tensor("pool_in", [H, MC], F32)
    pool_out = nc.dram_tensor("pool_out", [H, MC], F32, addr_space="Shared")

    rg = [list(range(CORES))]

    with tile.TileContext(nc) as tc:
        with (
            tc.tile_pool(name="const", bufs=1) as cpool,
            tc.tile_pool(name="idx", bufs=8) as ipool,
            tc.tile_pool(name="meta", bufs=8) as mpool,
            tc.tile_pool(name="gath", bufs=8) as dpool,
            tc.tile_pool(name="oneh", bufs=6) as opool,
            tc.tile_pool(name="aggs", bufs=2) as apool,
            tc.tile_pool(name="work", bufs=4) as wpool,
            tc.tile_pool(name="pa", bufs=1, space="PSUM") as pa,
            tc.tile_pool(name="po", bufs=2, space="PSUM") as po,
            tc.tile_pool(name="pb", bufs=1, space="PSUM") as pb,
            tc.tile_pool(name="pp", bufs=1, space="PSUM") as pp,
        ):
            # ---- constants
            id_f32 = cpool.tile([128, 128], F32)
            make_identity(nc, id_f32[:])
            id_bf = cpool.tile([128, 128], BF16)
            nc.vector.tensor_copy(id_bf[:], id_f32[:])

            zl = cpool.tile([128, 128], BF16, tag="zl")
            nc.vector.memset(zl[:], 0.0)
            zr = cpool.tile([128, NREL * 128], BF16, tag="zr")
            nc.vector.memset(zr[:], 0.0)

            iota_i = cpool.tile([128, 128], I32)
            nc.gpsimd.iota(iota_i[:], pattern=[[1, 128]], base=0,
                           channel_multiplier=0)
            iota_f = cpool.tile([128, 128], BF16)
            nc.vector.tensor_copy(iota_f[:], iota_i[:])

            w65 = cpool.tile([IN + 1, H], BF16)
            nc.sync.dma_start(w65[:], p["W65"][:, :])
            wl = {}
            for li, name in ((1, "Wl1"), (2, "Wl2")):
                for r in range(NREL):
                    t = cpool.tile([H, H], BF16, tag=f"wl{li}{r}")
                    nc.sync.dma_start(t[:], p[name][r, :, :])
                    wl[(li, r)] = t
            wm = {}
            for name in ("Wm1", "Wm2"):
                t = cpool.tile([H, H], BF16, tag=name)
                nc.sync.dma_start(t[:], p[name][:, :])
                wm[name] = t
            wm3 = cpool.tile([H, C], BF16)
            nc.sync.dma_start(wm3[:], p["Wm3"][:, :])
            biases = {}
            for name in ("B1", "B2", "bm1", "bm2"):
                t = cpool.tile([H, 1], F32, tag=name)
                nc.sync.dma_start(t[:], p[name][:, :])
                biases[name] = t
            bm3 = cpool.tile([C, 1], F32)
            nc.sync.dma_start(bm3[:], p["bm3"][:, :])

            # ---- phase 0: h0 = relu(x @ W_in + b_in) for this core's shard
            with tc.tile_pool(name="ph0", bufs=1) as hpool:
                xs = hpool.tile([IN + 1, SHARD], BF16)
                nc.sync.dma_start(xs[:], p["xTs"][:, :])
                for t in range(NBLK):
                    t0 = t * 128
                    tw = min(128, SHARD - t0)
                    ps = po.tile([128, H], F32, tag="mm")
                    nc.tensor.matmul(ps[:tw, :], lhsT=xs[:, t0:t0 + tw],
                                     rhs=w65[:], start=True, stop=True)
                    hb = wpool.tile([128, H], BF16, tag="h0out")
                    if t % 2 == 0:
                        nc.scalar.activation(hb[:tw, :], ps[:tw, :],
                                             mybir.ActivationFunctionType.Relu)
                    else:
                        nc.vector.tensor_scalar_max(hb[:tw, :], ps[:tw, :], 0.0)
                    nc.sync.dma_start(h0_shard[t0:t0 + tw, :], hb[:tw, :])

            nc.gpsimd.collective_compute(
                "AllGather", mybir.AluOpType.bypass, replica_groups=rg,
                ins=[h0_shard[:, :]], outs=[h0_full[:, :]])

            tables = [h0_full[g * GRP:(g + 1) * GRP, :] for g in range(NGRP)]
            pooled = pp.tile([H, MC], F32, tag="pooled")

            # ---- conv1 + fused pooling
            qi = 0  # running call index for queue round-robin / engine split
            for sb in sbs:
                agg = {}
                for call in sb["calls"]:
                    g, kc, q0, col0 = call["g"], call["kc"], call["q0"], call["col0"]
                    it = ipool.tile([128, KMAX * 8], I16, tag="it")
                    nc.sync.dma_start(it[:, :kc * 8],
                                      p["IDX"][:, col0:col0 + kc * 8])
                    st = dpool.tile([128, KMAX, H], BF16, tag="st")
                    nc.gpsimd.dma_gather(
                        st[:, :kc, :], tables[g], it[:, :kc * 8],
                        kc * 128, kc * 128, H, single_packet=False,
                        queue_num=qi % 4)
                    dslt = mpool.tile([128, KMAX], BF16, tag="dsl")
                    nc.scalar.dma_start(dslt[:, :kc], p["DSL"][:, q0:q0 + kc])
                    wct = mpool.tile([128, KMAX], BF16, tag="wc")
                    nc.scalar.dma_start(wct[:, :kc], p["WC"][:, q0:q0 + kc])
                    # one-hot build and edge-weight scale, both on DVE
                    oh = opool.tile([128, KMAX, 128], BF16, tag="oh")
                    iview = iota_f[:, :].rearrange("p (o f) -> p o f", o=1)\
                        .broadcast_to([128, kc, 128])
                    nc.vector.tensor_tensor(
                        out=oh[:, :kc, :],
                        in0=dslt[:, :kc].to_broadcast([128, kc, 128]),
                        in1=iview, op=mybir.AluOpType.is_equal)
                    nc.vector.tensor_tensor(
                        out=oh[:, :kc, :], in0=oh[:, :kc, :],
                        in1=wct[:, :kc].to_broadcast([128, kc, 128]),
                        op=mybir.AluOpType.mult)
                    for (i, b, r, first, last) in call["mms"]:
                        if b not in agg:
                            agg[b] = pa.tile([128, NREL * 128], F32,
                                             name=f"agg{b % SBS}",
                                             tag=f"agg{b % SBS}")
                            # start=True zeroes the whole PSUM bank, so do it
                            # once with a full-width zero matmul; the real
                            # slice matmuls then all accumulate.
                            nc.tensor.matmul(agg[b][:], lhsT=zl[:], rhs=zr[:],
                                             start=True, stop=False)
                        nc.tensor.matmul(agg[b][:, r * 128:(r + 1) * 128],
                                         lhsT=st[:, i, :], rhs=oh[:, i, :],
                                         start=False, stop=last)
                    qi += 1
                # superblock epilogue
                for b in sb["blocks"]:
                    rows = min(128, SHARD - b * 128)
                    ag4 = apool.tile([128, NREL * 128], BF16, tag=f"as{b % 2}")
                    if b in agg:
                        nc.scalar.activation(
                            ag4[:], agg[b][:],
                            mybir.ActivationFunctionType.Copy)
                    else:
                        nc.vector.memset(ag4[:], 0.0)
                    for (eb, er) in sb["empty"]:
                        if eb == b and (b in agg):
                            nc.vector.memset(
                                ag4[:, er * 128:(er + 1) * 128], 0.0)
                    out2 = po.tile([128, 128], F32, tag="mm")
                    for r in range(NREL):
                        nc.tensor.matmul(
                            out2[:], lhsT=wl[(1, r)][:],
                            rhs=ag4[:, r * 128:(r + 1) * 128],
                            start=(r == 0), stop=(r == NREL - 1))
                    t1 = wpool.tile([128, 128], BF16, tag="t1")
                    nc.scalar.activation(
                        t1[:], out2[:],
                        mybir.ActivationFunctionType.Relu,
                        bias=biases["B1"][:, :])
                    tb_ = pb.tile([128, 128], BF16, tag="tb")
                    nc.tensor.transpose(tb_[:], t1[:], id_bf[:])
                    t2 = wpool.tile([128, 128], BF16, tag="t2")
                    nc.scalar.activation(
                        t2[:], tb_[:],
                        mybir.ActivationFunctionType.Copy)
                    mb = wpool.tile([128, MC], BF16, tag="mb")
                    nc.scalar.dma_start(
                        mb[:rows, :], p["M"][b * 128:b * 128 + rows, :])
                    nc.tensor.matmul(pooled[:], lhsT=t2[:rows, :],
                                     rhs=mb[:rows, :],
                                     start=(b == 0), stop=(b == NBLK - 1))

            # ---- pooled partial sums -> AllReduce -> W2 contraction -> head
            psb = wpool.tile([H, MC], F32, tag="psb")
            nc.scalar.activation(psb[:], pooled[:],
                                 mybir.ActivationFunctionType.Copy)
            nc.sync.dma_start(pool_in[:, :], psb[:])
            nc.gpsimd.collective_compute(
                "AllReduce", mybir.AluOpType.add, replica_groups=rg,
                ins=[pool_in[:, :]], outs=[pool_out[:, :]])
            pool_f32 = wpool.tile([H, MC], F32, tag="pool_f32")
            nc.sync.dma_start(pool_f32[:], pool_out[:, :])
            pool_f = wpool.tile([H, MC], BF16, tag="pool_f")
            nc.vector.tensor_copy(pool_f[:], pool_f32[:])

            z0p = po.tile([H, G], F32, tag="mm")
            for r in range(NREL):
                nc.tensor.matmul(z0p[:], lhsT=wl[(2, r)][:],
                                 rhs=pool_f[:, r * G:(r + 1) * G],
                                 start=(r == 0), stop=(r == NREL - 1))
            # + B2 (conv2 bias, linear through the mean), cast to bf16
            zb = wpool.tile([H, G], BF16, tag="zb")
            nc.vector.tensor_scalar_add(zb[:], z0p[:], biases["B2"][:, :])

            z1p = po.tile([H, G], F32, tag="mm")
            nc.tensor.matmul(z1p[:], lhsT=wm["Wm1"][:], rhs=zb[:],
                             start=True, stop=True)
            z1 = wpool.tile([H, G], BF16, tag="z1")
            nc.scalar.activation(z1[:], z1p[:],
                                 mybir.ActivationFunctionType.Relu,
                                 bias=biases["bm1"][:, :])
            z2p = po.tile([H, G], F32, tag="mm")
            nc.tensor.matmul(z2p[:], lhsT=wm["Wm2"][:], rhs=z1[:],
                             start=True, stop=True)
            z2 = wpool.tile([H, G], BF16, tag="z2")
            nc.scalar.activation(z2[:], z2p[:],
                                 mybir.ActivationFunctionType.Relu,
                                 bias=biases["bm2"][:, :])
            z3p = po.tile([C, G], F32, tag="mm")
            nc.tensor.matmul(z3p[:], lhsT=wm3[:], rhs=z2[:],
                             start=True, stop=True)
            z3 = wpool.tile([C, G], F32, tag="z3")
            nc.vector.tensor_scalar_add(z3[:], z3p[:], bm3[:, :])
            nc.sync.dma_start(out_ext[:, :], z3[:])

    nc.compile()
    return nc


# ---------------------------------------------------------------------------
# entry point
# ---------------------------------------------------------------------------

_CACHE = {}


def kernel(x, src, dst, graph_ids, W_in, b_in, W1, b1, W2, b2,
           Wm1, bm1, Wm2, bm2, Wm3, bm3):
    x = np.asarray(x)
    key = (int(np.asarray(src).sum()) ^ int(np.asarray(dst).sum()),
           int(np.asarray(graph_ids).sum()))
    if key not in _CACHE:
        plan = _plan(src, dst, graph_ids)
        nc = _build(plan)
        _CACHE[key] = (plan, nc)
    plan, nc = _CACHE[key]

    xT = np.concatenate([np.asarray(x).T, np.ones((1, N), np.float32)], axis=0)
    w65 = np.concatenate([np.asarray(W_in), np.asarray(b_in)[None, :]], axis=0)

    def bf(a):
        return np.ascontiguousarray(np.asarray(a), dtype=np.float32).astype(bfloat16)

    def col(a):
        return np.ascontiguousarray(np.asarray(a, np.float32).reshape(-1, 1))

    xTb = bf(xT)
    in_maps = []
    for c in range(CORES):
        in_maps.append({
            "xTs": np.ascontiguousarray(xTb[:, c * SHARD:(c + 1) * SHARD]),
            "W65": bf(w65),
            "Wl1": bf(W1),
            "Wl2": bf(W2),
            "Wm1": bf(Wm1),
            "Wm2": bf(Wm2),
            "Wm3": bf(Wm3),
            "B1": col(np.asarray(b1, np.float32).sum(axis=0)),
            "B2": col(np.asarray(b2, np.float32).sum(axis=0)),
            "bm1": col(bm1),
            "bm2": col(bm2),
            "bm3": col(bm3),
            "IDX": np.ascontiguousarray(plan["IDX"][c]),
            "DSL": np.ascontiguousarray(plan["DSL"][c]),
            "WC": np.ascontiguousarray(plan["WC"][c]),
            "M": np.ascontiguousarray(plan["M"][c]),
        })

    res = bass_utils.run_bass_kernel_spmd(nc, in_maps, list(range(CORES)))
    global LAST_EXEC_NS
    LAST_EXEC_NS = res.exec_time_ns
    out = np.asarray(res.results[0]["out"], np.float32)  # [C, G]
    return np.ascontiguousarray(out.T)                   # [G, C]


LAST_EXEC_NS = None


if __name__ == "__main__":
    import reference
    import jax
    with jax.default_device(jax.devices("cpu")[0]):
        inp = {k: np.asarray(v) for k, v in reference.setup_inputs().items()}
        exp = np.asarray(reference.reference(**{k: v for k, v in inp.items()}))
    act = kernel(**inp)
    rel = np.linalg.norm(act - exp) / np.linalg.norm(exp)
    print("Relative error:", rel)


# revision 18
# speedup vs baseline: 2.8340x; 1.9332x over previous
"""RGCN (2x hetero GraphConv + mean-pool + MLP) on 8 TRN2 NeuronCores — v3.

Key structure (vs v2 baseline):
- Layer 2 + mean-pooling are algebraically fused into a dense matmul: with no
  relu after conv2 and pooling linear, pooled[g] only needs
  M[n,(r,g)] = sum_{e in r: src=n, gid(dst)=g} w_e / c_g (host-built graph
  metadata). Per dst block: pool += h1_block^T @ M_block. This removes the
  second gather/scatter pass and the h1 AllGather entirely.
- Layer 1 keeps the dst-sharded gather/one-hot-scatter design but with
  relation-pure 128-edge chunks (cells keyed (block, relation, src-group)) so
  one-hots are 128 wide (4x less PE/DVE work than the 512-wide merged form).
- dma_gather calls round-robin across 4 SWDGE queues; each queue maps to a
  different GpSimd Q7 cpu pair, so descriptor generation (the old wall at
  ~9.3 ns/row) runs 4-wide (~2.1 ns/row).
- Per-edge normalization w_e = rsqrt(deg_out_r[src])*rsqrt(deg_in_r[dst]) is a
  tensor_tensor mult of the gathered rows by a broadcast weight column;
  split between DVE and GpSimd to balance engine load.

SPMD: identical instruction stream on all 8 cores; per-core variation lives in
IDX/DSLWC/M tensors. Chunk counts per cell are the max over cores.
"""

import numpy as np
from ml_dtypes import bfloat16

import concourse.bass as bass
import concourse.bacc as bacc
import concourse.mybir as mybir
import concourse.tile as tile
from concourse import bass_utils
from concourse.masks import make_identity

F32 = mybir.dt.float32
BF16 = mybir.dt.bfloat16
I16 = mybir.dt.int16
I32 = mybir.dt.int32

# problem constants (hardcoded per spec)
N, E, NREL, G, IN, H, C = 100000, 400000, 4, 64, 64, 128, 2
CORES = 8
SHARD = N // CORES            # 12500
NBLK = (SHARD + 127) // 128   # 98
GRP = 25000                   # src rows per gather group (int16 idx limit)
NGRP = N // GRP               # 4
SBS = 4                       # dst blocks per superblock (PSUM banks)
KMAX = 16                     # max 128-edge chunks per gather call
MC = NREL * G                 # pooling matrix columns: 256


# ---------------------------------------------------------------------------
# host-side planning: pure graph-structure metadata (indices, degrees, layout)
# ---------------------------------------------------------------------------

def _plan(src, dst, graph_ids):
    src = np.asarray(src).astype(np.int64)
    dst = np.asarray(dst).astype(np.int64)
    gid = np.asarray(graph_ids).astype(np.int64)

    # folded normalization: w_e = rsqrt(deg_in[dst]) * rsqrt(deg_out[src])
    w_all = np.empty((NREL, E), np.float32)
    for r in range(NREL):
        do = np.maximum(np.bincount(src[r], minlength=N), 1.0)
        di = np.maximum(np.bincount(dst[r], minlength=N), 1.0)
        w_all[r] = (1.0 / np.sqrt(do[src[r]]) / np.sqrt(di[dst[r]])).astype(np.float32)

    # ---- node relabeling: permute nodes within each src-group so that the
    # per-(block, relation, group) in-degree is balanced across blocks and
    # cores (pushes chunks/cell toward 1 and minimizes gather padding).
    # perm[slot] = original node id at relabeled slot; inv[n] = slot of n.
    deg = np.zeros((N, NREL * NGRP), np.int32)
    for r in range(NREL):
        gs = src[r] // GRP
        np.add.at(deg, (dst[r], r * NGRP + gs), 1)
    perm = np.empty(N, np.int64)
    rng_bal = np.random.default_rng(12345)
    NB_G = 2 * NBLK                      # blocks per group (2 cores)
    NA = 32                              # absorber blocks per group
    QB = 126.0                           # per-coord quota for regular blocks
    for grp in range(NGRP):
        nodes = np.arange(grp * GRP, (grp + 1) * GRP)
        v = deg[nodes].astype(np.float64)          # [GRP, 16]
        order = np.argsort(-v.sum(axis=1), kind="stable")
        nodes = nodes[order]
        v = v[order]
        ncap = np.full(NB_G, 128, np.int64)
        ncap[NA] = ncap[NA + 1] = SHARD - (NBLK - 1) * 128  # ragged pair
        S = np.zeros((NB_G, NREL * NGRP))
        fill = np.zeros(NB_G, np.int64)
        assign = np.empty(GRP, np.int64)
        # phase 1: heaviest nodes snake-dealt into absorber blocks 0..NA-1
        # (fill only 3/4; the rest of absorber capacity is the relief valve
        # for phase-2 nodes that fit no regular block)
        nheavy = NA * 96
        for j in range(nheavy):
            k = j % (2 * NA)
            bsel = k if k < NA else 2 * NA - 1 - k
            assign[j] = bsel
            S[bsel] += v[j]
            fill[bsel] += 1
        # phase 2: folded order (heavy/light interleaved) + worst-fit:
        # every regular block grows evenly in node count and coord mass.
        K = 32
        rest = np.arange(nheavy, GRP)
        folded = np.empty_like(rest)
        folded[0::2] = rest[: (len(rest) + 1) // 2]
        folded[1::2] = rest[(len(rest) + 1) // 2:][::-1]
        cand_all = NA + 2 + rng_bal.integers(0, NB_G - NA - 2,
                                             size=(GRP, K))
        for j in folded:
            cands = cand_all[j]
            room = fill[cands] < ncap[cands]
            if not room.any():
                cands = np.where(fill < ncap)[0]
                room = np.ones(len(cands), bool)
            Sv = S[cands] + v[j]
            scq = np.where(room, Sv.max(axis=1), np.inf)
            bsel = int(cands[int(np.argmin(scq))])
            assign[j] = bsel
            S[bsel] += v[j]
            fill[bsel] += 1
        # layout: per half (core): regular blocks first, ragged at slot
        # NBLK-1-NA, absorbers at the last NA slots -- aligned across cores.
        reg = [k for k in range(NA + 2, NB_G)]
        key = np.lexsort(S[reg].T)
        reg = [reg[k] for k in key]
        halves = [[], []]
        for k, bk in enumerate(reg):
            halves[k % 2].append(bk)
        for half in range(2):
            # 92 regular + 1 ragged + 5 absorbers; absorber slots align
            # across all cores (same positions), so their 2-chunk cells max
            # together instead of polluting regular slots.
            blocks = halves[half] + [NA + half]
            blocks += [a for a in range(NA) if a % 2 == half]
            cc = 2 * grp + half
            flat = []
            for bk in blocks:
                mem = nodes[assign == bk]
                flat.extend(mem.tolist())
            assert len(flat) == SHARD, (cc, len(flat), len(blocks))
            perm[cc * SHARD: (cc + 1) * SHARD] = np.array(flat, np.int64)
    inv = np.empty(N, np.int64)
    inv[perm] = np.arange(N)

    # relabel edge endpoints into slot space; src groups unchanged by
    # construction (perm permutes within each GRP range)
    src = inv[src]
    dst = inv[dst]
    # cells keyed (block, relation, group); count per core
    NC_CELL = NBLK * NREL * NGRP
    cnt = np.zeros((CORES, NC_CELL), np.int64)
    cell_data = [[None] * NC_CELL for _ in range(CORES)]
    for c in range(CORES):
        for r in range(NREL):
            local = dst[r] - c * SHARD
            m = (local >= 0) & (local < SHARD)
            es = np.nonzero(m)[0]
            loc = local[es]
            b_arr = loc // 128
            g_arr = src[r][es] // GRP
            key = (b_arr * NREL + r) * NGRP + g_arr
            order = np.argsort(key, kind="stable")
            es, loc, key = es[order], loc[order], key[order]
            bounds = np.searchsorted(key, np.arange(NC_CELL + 1))
            si_all = (src[r][es] % GRP).astype(np.int16)
            dl_all = (loc % 128).astype(np.int16)
            wv_all = w_all[r][es].astype(np.float32)
            for b in range(NBLK):
                for g in range(NGRP):
                    cell = (b * NREL + r) * NGRP + g
                    s0, s1 = bounds[cell], bounds[cell + 1]
                    if s0 == s1:
                        continue
                    cell_data[c][cell] = (si_all[s0:s1], dl_all[s0:s1],
                                          wv_all[s0:s1])
                    cnt[c, cell] = s1 - s0

    nch = -(-cnt.max(axis=0) // 128)          # [NC_CELL] chunks per cell

    # call layout: per superblock, per group, chunks from cells
    # (b in sb, r in 0..3); split into calls of <= KMAX chunks.
    sbs = []
    q = 0       # global chunk counter (DSL/WC columns)
    col = 0     # IDX int16 column counter
    for s0 in range(0, NBLK, SBS):
        blocks = list(range(s0, min(s0 + SBS, NBLK)))
        calls = []
        # first/last bookkeeping per (b, r) across the sb
        br_chunks = {}
        for g in range(NGRP):
            descs = []
            tail = []
            for b in blocks:
                for r in range(NREL):
                    cell = (b * NREL + r) * NGRP + g
                    for k in range(int(nch[cell])):
                        if k == 0:
                            descs.append((b, r, g, k, cell))
                        else:
                            tail.append((b, r, g, k, cell))
            # overflow chunks last (likely empty on most cores -> their
            # trailing -1 idxs are trimmed by the Q7 gather at runtime);
            # order by descending mean count so emptier chunks go later
            tail.sort(key=lambda d: -int(cnt[:, d[4]].sum()))
            descs += tail
            for i0 in range(0, len(descs), KMAX):
                part = descs[i0:i0 + KMAX]
                call = dict(g=g, kc=len(part), q0=q, col0=col, part=part)
                calls.append(call)
                for i, (b, r, g_, k, cell) in enumerate(part):
                    br_chunks.setdefault((b, r), []).append(
                        (len(calls) - 1, i))
                q += len(part)
                col += len(part) * 8
        # flags: stop on the last matmul targeting block b within the sb
        b_last = {}
        for ci, call in enumerate(calls):
            for i, (b, r, g_, k, cell) in enumerate(call["part"]):
                b_last[b] = (ci, i)
        for ci, call in enumerate(calls):
            mms = []
            for i, (b, r, g_, k, cell) in enumerate(call["part"]):
                last = b_last.get(b) == (ci, i)
                mms.append((i, b, r, False, last))
            call["mms"] = mms
        empty = [(b, r) for b in blocks for r in range(NREL)
                 if (b, r) not in br_chunks]
        sbs.append(dict(blocks=blocks, calls=calls, empty=empty))
    NCH = q
    NIDXCOL = col

    # chunk data arrays
    IDX = np.zeros((CORES, 16, NIDXCOL), np.int16)
    DSL = np.full((CORES, 128, NCH), -1.0, np.float32)
    WC = np.zeros((CORES, 128, NCH), np.float32)
    for c in range(CORES):
        for sb in sbs:
            for call in sb["calls"]:
                kc = call["kc"]
                last_real = -1  # last idx slot (within call) holding a real edge
                for i, (b, r, g, k, cell) in enumerate(call["part"]):
                    data = cell_data[c][cell]
                    if data is None:
                        continue
                    si, dl, wv = data
                    v0, v1 = k * 128, min((k + 1) * 128, len(si))
                    take = v1 - v0
                    if take <= 0:
                        continue
                    qq = call["q0"] + i
                    nn = i * 128 + np.arange(take)
                    IDX[c, nn % 16, call["col0"] + nn // 16] = si[v0:v1]
                    DSL[c, :take, qq] = dl[v0:v1]
                    WC[c, :take, qq] = wv[v0:v1]
                    last_real = max(last_real, i * 128 + take - 1)
                # (trailing -1 trim disabled: caused device crash)
                del last_real

    # pooling matrix M[slot, r*G + g] = sum_{e in r: src-slot, gid(dst)=g} w/c_g
    # (src/dst already in slot space; gid must be looked up via perm)
    cnt_g = np.maximum(np.bincount(gid, minlength=G), 1.0)
    M = np.zeros((N, MC), np.float32)
    for r in range(NREL):
        gd = gid[perm[dst[r]]]
        np.add.at(M, (src[r], r * G + gd), w_all[r] / cnt_g[gd])
    Mpad = np.zeros((CORES, NBLK * 128, MC), np.float32)
    for c in range(CORES):
        Mpad[c, :SHARD] = M[c * SHARD:(c + 1) * SHARD]

    return dict(sbs=sbs, NCH=NCH, NIDXCOL=NIDXCOL,
                IDX=np.tile(IDX, (1, 8, 1)),
                DSL=DSL.astype(bfloat16), WC=WC.astype(bfloat16),
                M=Mpad.astype(bfloat16), perm=perm)


# ---------------------------------------------------------------------------
# device program
# ---------------------------------------------------------------------------

def _build(plan):
    sbs = plan["sbs"]
    NCH = plan["NCH"]
    NIDXCOL = plan["NIDXCOL"]

    nc = bacc.Bacc(None, target_bir_lowering=False, num_devices=CORES,
                   num_swdge_queues=4)

    p = {}
    p["xTs"] = nc.declare_dram_parameter("xTs", [IN + 1, SHARD], BF16, isOutput=False)
    p["W65"] = nc.declare_dram_parameter("W65", [IN + 1, H], BF16, isOutput=False)
    p["Wl1"] = nc.declare_dram_parameter("Wl1", [NREL, H, H], BF16, isOutput=False)
    p["Wl2"] = nc.declare_dram_parameter("Wl2", [NREL, H, H], BF16, isOutput=False)
    p["Wm1"] = nc.declare_dram_parameter("Wm1", [H, H], BF16, isOutput=False)
    p["Wm2"] = nc.declare_dram_parameter("Wm2", [H, H], BF16, isOutput=False)
    p["Wm3"] = nc.declare_dram_parameter("Wm3", [H, C], BF16, isOutput=False)
    p["B1"] = nc.declare_dram_parameter("B1", [H, 1], F32, isOutput=False)
    p["B2"] = nc.declare_dram_parameter("B2", [H, 1], F32, isOutput=False)
    p["bm1"] = nc.declare_dram_parameter("bm1", [H, 1], F32, isOutput=False)
    p["bm2"] = nc.declare_dram_parameter("bm2", [H, 1], F32, isOutput=False)
    p["bm3"] = nc.declare_dram_parameter("bm3", [C, 1], F32, isOutput=False)
    p["IDX"] = nc.declare_dram_parameter("IDX", [128, NIDXCOL], I16, isOutput=False)
    p["DSL"] = nc.declare_dram_parameter("DSL", [128, NCH], BF16, isOutput=False)
    p["WC"] = nc.declare_dram_parameter("WC", [128, NCH], BF16, isOutput=False)
    p["M"] = nc.declare_dram_parameter("M", [NBLK * 128, MC], BF16, isOutput=False)
    out_ext = nc.declare_dram_parameter("out", [C, G], F32, isOutput=True)

    h0_shard = nc.dram_tensor("h0_shard", [SHARD, H], BF16)
    h0_full = nc.dram_tensor("h0_full", [N, H], BF16, addr_space="Shared")
    pool_in = nc.dram_tensor("pool_in", [H, MC], F32)
    pool_out = nc.dram_tensor("pool_out", [H, MC], F32, addr_space="Shared")

    rg = [list(range(CORES))]

    with tile.TileContext(nc) as tc:
        with (
            tc.tile_pool(name="const", bufs=1) as cpool,
            tc.tile_pool(name="idx", bufs=12) as ipool,
            tc.tile_pool(name="meta", bufs=12) as mpool,
            tc.tile_pool(name="gath", bufs=12) as dpool,
            tc.tile_pool(name="oneh", bufs=10) as opool,
            tc.tile_pool(name="aggs", bufs=2) as apool,
            tc.tile_pool(name="work", bufs=4) as wpool,
            tc.tile_pool(name="pa", bufs=1, space="PSUM") as pa,
            tc.tile_pool(name="po", bufs=2, space="PSUM") as po,
            tc.tile_pool(name="pb", bufs=1, space="PSUM") as pb,
            tc.tile_pool(name="pp", bufs=1, space="PSUM") as pp,
        ):
            # ---- constants
            id_f32 = cpool.tile([128, 128], F32)
            make_identity(nc, id_f32[:])
            id_bf = cpool.tile([128, 128], BF16)
            nc.vector.tensor_copy(id_bf[:], id_f32[:])

            zl = cpool.tile([128, 128], BF16, tag="zl")
            nc.vector.memset(zl[:], 0.0)
            zr = cpool.tile([128, NREL * 128], BF16, tag="zr")
            nc.vector.memset(zr[:], 0.0)

            iota_i = cpool.tile([128, 128], I32)
            nc.gpsimd.iota(iota_i[:], pattern=[[1, 128]], base=0,
                           channel_multiplier=0)
            iota_f = cpool.tile([128, 128], BF16)
            nc.vector.tensor_copy(iota_f[:], iota_i[:])

            w65 = cpool.tile([IN + 1, H], BF16)
            nc.sync.dma_start(w65[:], p["W65"][:, :])
            wl = {}
            for li, name in ((1, "Wl1"), (2, "Wl2")):
                for r in range(NREL):
                    t = cpool.tile([H, H], BF16, tag=f"wl{li}{r}")
                    nc.sync.dma_start(t[:], p[name][r, :, :])
                    wl[(li, r)] = t
            wm = {}
            for name in ("Wm1", "Wm2"):
                t = cpool.tile([H, H], BF16, tag=name)
                nc.sync.dma_start(t[:], p[name][:, :])
                wm[name] = t
            wm3 = cpool.tile([H, C], BF16)
            nc.sync.dma_start(wm3[:], p["Wm3"][:, :])
            biases = {}
            for name in ("B1", "B2", "bm1", "bm2"):
                t = cpool.tile([H, 1], F32, tag=name)
                nc.sync.dma_start(t[:], p[name][:, :])
                biases[name] = t
            bm3 = cpool.tile([C, 1], F32)
            nc.sync.dma_start(bm3[:], p["bm3"][:, :])

            # ---- phase 0: h0 = relu(x @ W_in + b_in) for this core's shard
            with tc.tile_pool(name="ph0", bufs=1) as hpool:
                xs = hpool.tile([IN + 1, SHARD], BF16)
                nc.sync.dma_start(xs[:], p["xTs"][:, :])
                for t in range(NBLK):
                    t0 = t * 128
                    tw = min(128, SHARD - t0)
                    ps = po.tile([128, H], F32, tag="mm")
                    nc.tensor.matmul(ps[:tw, :], lhsT=xs[:, t0:t0 + tw],
                                     rhs=w65[:], start=True, stop=True)
                    hb = wpool.tile([128, H], BF16, tag="h0out")
                    if t % 2 == 0:
                        nc.scalar.activation(hb[:tw, :], ps[:tw, :],
                                             mybir.ActivationFunctionType.Relu)
                    else:
                        nc.vector.tensor_scalar_max(hb[:tw, :], ps[:tw, :], 0.0)
                    nc.sync.dma_start(h0_shard[t0:t0 + tw, :], hb[:tw, :])

            nc.gpsimd.collective_compute(
                "AllGather", mybir.AluOpType.bypass, replica_groups=rg,
                ins=[h0_shard[:, :]], outs=[h0_full[:, :]])

            tables = [h0_full[g * GRP:(g + 1) * GRP, :] for g in range(NGRP)]
            pooled = pp.tile([H, MC], F32, tag="pooled")

            # ---- conv1 + fused pooling
            qi = 0  # running call index for queue round-robin / engine split
            for sb in sbs:
                agg = {}
                for call in sb["calls"]:
                    g, kc, q0, col0 = call["g"], call["kc"], call["q0"], call["col0"]
                    it = ipool.tile([128, KMAX * 8], I16, tag="it")
                    nc.sync.dma_start(it[:, :kc * 8],
                                      p["IDX"][:, col0:col0 + kc * 8])
                    st = dpool.tile([128, KMAX, H], BF16, tag="st")
                    nc.gpsimd.dma_gather(
                        st[:, :kc, :], tables[g], it[:, :kc * 8],
                        kc * 128, kc * 128, H, single_packet=False,
                        queue_num=qi % 4)
                    dslt = mpool.tile([128, KMAX], BF16, tag="dsl")
                    nc.scalar.dma_start(dslt[:, :kc], p["DSL"][:, q0:q0 + kc])
                    wct = mpool.tile([128, KMAX], BF16, tag="wc")
                    nc.scalar.dma_start(wct[:, :kc], p["WC"][:, q0:q0 + kc])
                    # one-hot build and edge-weight scale, both on DVE
                    oh = opool.tile([128, KMAX, 128], BF16, tag="oh")
                    iview = iota_f[:, :].rearrange("p (o f) -> p o f", o=1)\
                        .broadcast_to([128, kc, 128])
                    nc.vector.tensor_tensor(
                        out=oh[:, :kc, :],
                        in0=dslt[:, :kc].to_broadcast([128, kc, 128]),
                        in1=iview, op=mybir.AluOpType.is_equal)
                    nc.vector.tensor_tensor(
                        out=oh[:, :kc, :], in0=oh[:, :kc, :],
                        in1=wct[:, :kc].to_broadcast([128, kc, 128]),
                        op=mybir.AluOpType.mult)
                    for (i, b, r, first, last) in call["mms"]:
                        if b not in agg:
                            agg[b] = pa.tile([128, NREL * 128], F32,
                                             name=f"agg{b % SBS}",
                                             tag=f"agg{b % SBS}")
                            # start=True zeroes the whole PSUM bank, so do it
                            # once with a full-width zero matmul; the real
                            # slice matmuls then all accumulate.
                            nc.tensor.matmul(agg[b][:], lhsT=zl[:], rhs=zr[:],
                                             start=True, stop=False)
                        nc.tensor.matmul(agg[b][:, r * 128:(r + 1) * 128],
                                         lhsT=st[:, i, :], rhs=oh[:, i, :],
                                         start=False, stop=last)
                    qi += 1
                # superblock epilogue
                for b in sb["blocks"]:
                    rows = min(128, SHARD - b * 128)
                    ag4 = apool.tile([128, NREL * 128], BF16, tag=f"as{b % 2}")
                    if b in agg:
                        nc.scalar.activation(
                            ag4[:], agg[b][:],
                            mybir.ActivationFunctionType.Copy)
                    else:
                        nc.vector.memset(ag4[:], 0.0)
                    for (eb, er) in sb["empty"]:
                        if eb == b and (b in agg):
                            nc.vector.memset(
                                ag4[:, er * 128:(er + 1) * 128], 0.0)
                    out2 = po.tile([128, 128], F32, tag="mm")
                    for r in range(NREL):
                        nc.tensor.matmul(
                            out2[:], lhsT=wl[(1, r)][:],
                            rhs=ag4[:, r * 128:(r + 1) * 128],
                            start=(r == 0), stop=(r == NREL - 1))
                    t1 = wpool.tile([128, 128], BF16, tag="t1")
                    nc.scalar.activation(
                        t1[:], out2[:],
                        mybir.ActivationFunctionType.Relu,
                        bias=biases["B1"][:, :])
                    tb_ = pb.tile([128, 128], BF16, tag="tb")
                    nc.tensor.transpose(tb_[:], t1[:], id_bf[:])
                    t2 = wpool.tile([128, 128], BF16, tag="t2")
                    nc.scalar.activation(
                        t2[:], tb_[:],
                        mybir.ActivationFunctionType.Copy)
                    mb = wpool.tile([128, MC], BF16, tag="mb")
                    nc.sync.dma_start(
                        mb[:rows, :], p["M"][b * 128:b * 128 + rows, :])
                    nc.tensor.matmul(pooled[:], lhsT=t2[:rows, :],
                                     rhs=mb[:rows, :],
                                     start=(b == 0), stop=(b == NBLK - 1))

            # ---- pooled partial sums -> AllReduce -> W2 contraction -> head
            psb = wpool.tile([H, MC], F32, tag="psb")
            nc.scalar.activation(psb[:], pooled[:],
                                 mybir.ActivationFunctionType.Copy)
            nc.sync.dma_start(pool_in[:, :], psb[:])
            nc.gpsimd.collective_compute(
                "AllReduce", mybir.AluOpType.add, replica_groups=rg,
                ins=[pool_in[:, :]], outs=[pool_out[:, :]])
            pool_f32 = wpool.tile([H, MC], F32, tag="pool_f32")
            nc.sync.dma_start(pool_f32[:], pool_out[:, :])
            pool_f = wpool.tile([H, MC], BF16, tag="pool_f")
            nc.vector.tensor_copy(pool_f[:], pool_f32[:])

            z0p = po.tile([H, G], F32, tag="mm")
            for r in range(NREL):
                nc.tensor.matmul(z0p[:], lhsT=wl[(2, r)][:],
                                 rhs=pool_f[:, r * G:(r + 1) * G],
                                 start=(r == 0), stop=(r == NREL - 1))
            # + B2 (conv2 bias, linear through the mean), cast to bf16
            zb = wpool.tile([H, G], BF16, tag="zb")
            nc.vector.tensor_scalar_add(zb[:], z0p[:], biases["B2"][:, :])

            z1p = po.tile([H, G], F32, tag="mm")
            nc.tensor.matmul(z1p[:], lhsT=wm["Wm1"][:], rhs=zb[:],
                             start=True, stop=True)
            z1 = wpool.tile([H, G], BF16, tag="z1")
            nc.scalar.activation(z1[:], z1p[:],
                                 mybir.ActivationFunctionType.Relu,
                                 bias=biases["bm1"][:, :])
            z2p = po.tile([H, G], F32, tag="mm")
            nc.tensor.matmul(z2p[:], lhsT=wm["Wm2"][:], rhs=z1[:],
                             start=True, stop=True)
            z2 = wpool.tile([H, G], BF16, tag="z2")
            nc.scalar.activation(z2[:], z2p[:],
                                 mybir.ActivationFunctionType.Relu,
                                 bias=biases["bm2"][:, :])
            z3p = po.tile([C, G], F32, tag="mm")
            nc.tensor.matmul(z3p[:], lhsT=wm3[:], rhs=z2[:],
                             start=True, stop=True)
            z3 = wpool.tile([C, G], F32, tag="z3")
            nc.vector.tensor_scalar_add(z3[:], z3p[:], bm3[:, :])
            nc.sync.dma_start(out_ext[:, :], z3[:])

    nc.compile()
    return nc


# ---------------------------------------------------------------------------
# entry point
# ---------------------------------------------------------------------------

_CACHE = {}


def kernel(x, src, dst, graph_ids, W_in, b_in, W1, b1, W2, b2,
           Wm1, bm1, Wm2, bm2, Wm3, bm3):
    x = np.asarray(x)
    key = (int(np.asarray(src).sum()) ^ int(np.asarray(dst).sum()),
           int(np.asarray(graph_ids).sum()))
    if key not in _CACHE:
        plan = _plan(src, dst, graph_ids)
        nc = _build(plan)
        _CACHE[key] = (plan, nc)
    plan, nc = _CACHE[key]

    xT = np.concatenate([np.asarray(x).T, np.ones((1, N), np.float32)], axis=0)
    xT = xT[:, plan["perm"]]
    w65 = np.concatenate([np.asarray(W_in), np.asarray(b_in)[None, :]], axis=0)

    def bf(a):
        return np.ascontiguousarray(np.asarray(a), dtype=np.float32).astype(bfloat16)

    def col(a):
        return np.ascontiguousarray(np.asarray(a, np.float32).reshape(-1, 1))

    xTb = bf(xT)
    in_maps = []
    for c in range(CORES):
        in_maps.append({
            "xTs": np.ascontiguousarray(xTb[:, c * SHARD:(c + 1) * SHARD]),
            "W65": bf(w65),
            "Wl1": bf(W1),
            "Wl2": bf(W2),
            "Wm1": bf(Wm1),
            "Wm2": bf(Wm2),
            "Wm3": bf(Wm3),
            "B1": col(np.asarray(b1, np.float32).sum(axis=0)),
            "B2": col(np.asarray(b2, np.float32).sum(axis=0)),
            "bm1": col(bm1),
            "bm2": col(bm2),
            "bm3": col(bm3),
            "IDX": np.ascontiguousarray(plan["IDX"][c]),
            "DSL": np.ascontiguousarray(plan["DSL"][c]),
            "WC": np.ascontiguousarray(plan["WC"][c]),
            "M": np.ascontiguousarray(plan["M"][c]),
        })

    res = bass_utils.run_bass_kernel_spmd(nc, in_maps, list(range(CORES)))
    global LAST_EXEC_NS
    LAST_EXEC_NS = res.exec_time_ns
    out = np.asarray(res.results[0]["out"], np.float32)  # [C, G]
    return np.ascontiguousarray(out.T)                   # [G, C]


LAST_EXEC_NS = None


if __name__ == "__main__":
    import reference
    import jax
    with jax.default_device(jax.devices("cpu")[0]):
        inp = {k: np.asarray(v) for k, v in reference.setup_inputs().items()}
        exp = np.asarray(reference.reference(**{k: v for k, v in inp.items()}))
    act = kernel(**inp)
    rel = np.linalg.norm(act - exp) / np.linalg.norm(exp)
    print("Relative error:", rel)


# revision 19
# speedup vs baseline: 3.1015x; 1.0944x over previous
"""RGCN (2x hetero GraphConv + mean-pool + MLP) on 8 TRN2 NeuronCores — v3.

Key structure (vs v2 baseline):
- Layer 2 + mean-pooling are algebraically fused into a dense matmul: with no
  relu after conv2 and pooling linear, pooled[g] only needs
  M[n,(r,g)] = sum_{e in r: src=n, gid(dst)=g} w_e / c_g (host-built graph
  metadata). Per dst block: pool += h1_block^T @ M_block. This removes the
  second gather/scatter pass and the h1 AllGather entirely.
- Layer 1 keeps the dst-sharded gather/one-hot-scatter design but with
  relation-pure 128-edge chunks (cells keyed (block, relation, src-group)) so
  one-hots are 128 wide (4x less PE/DVE work than the 512-wide merged form).
- dma_gather calls round-robin across 4 SWDGE queues; each queue maps to a
  different GpSimd Q7 cpu pair, so descriptor generation (the old wall at
  ~9.3 ns/row) runs 4-wide (~2.1 ns/row).
- Per-edge normalization w_e = rsqrt(deg_out_r[src])*rsqrt(deg_in_r[dst]) is a
  tensor_tensor mult of the gathered rows by a broadcast weight column;
  split between DVE and GpSimd to balance engine load.

SPMD: identical instruction stream on all 8 cores; per-core variation lives in
IDX/DSLWC/M tensors. Chunk counts per cell are the max over cores.
"""

import numpy as np
from ml_dtypes import bfloat16

import concourse.bass as bass
import concourse.bacc as bacc
import concourse.mybir as mybir
import concourse.tile as tile
from concourse import bass_utils
from concourse.masks import make_identity

F32 = mybir.dt.float32
BF16 = mybir.dt.bfloat16
I16 = mybir.dt.int16
I32 = mybir.dt.int32

# problem constants (hardcoded per spec)
N, E, NREL, G, IN, H, C = 100000, 400000, 4, 64, 64, 128, 2
CORES = 8
SHARD = N // CORES            # 12500
NBLK = (SHARD + 127) // 128   # 98
GRP = 25000                   # src rows per gather group (int16 idx limit)
NGRP = N // GRP               # 4
SBS = 4                       # dst blocks per superblock (PSUM banks)
KMAX = 16                     # max 128-edge chunks per gather call
MC = NREL * G                 # pooling matrix columns: 256


# ---------------------------------------------------------------------------
# host-side planning: pure graph-structure metadata (indices, degrees, layout)
# ---------------------------------------------------------------------------

def _plan(src, dst, graph_ids):
    src = np.asarray(src).astype(np.int64)
    dst = np.asarray(dst).astype(np.int64)
    gid = np.asarray(graph_ids).astype(np.int64)

    # folded normalization: w_e = rsqrt(deg_in[dst]) * rsqrt(deg_out[src])
    w_all = np.empty((NREL, E), np.float32)
    for r in range(NREL):
        do = np.maximum(np.bincount(src[r], minlength=N), 1.0)
        di = np.maximum(np.bincount(dst[r], minlength=N), 1.0)
        w_all[r] = (1.0 / np.sqrt(do[src[r]]) / np.sqrt(di[dst[r]])).astype(np.float32)

    # ---- node relabeling: permute nodes within each src-group so that the
    # per-(block, relation, group) in-degree is balanced across blocks and
    # cores (pushes chunks/cell toward 1 and minimizes gather padding).
    # perm[slot] = original node id at relabeled slot; inv[n] = slot of n.
    deg = np.zeros((N, NREL * NGRP), np.int32)
    for r in range(NREL):
        gs = src[r] // GRP
        np.add.at(deg, (dst[r], r * NGRP + gs), 1)
    perm = np.empty(N, np.int64)
    rng_bal = np.random.default_rng(12345)
    NB_G = 2 * NBLK                      # blocks per group (2 cores)
    NA = 32                              # absorber blocks per group
    QB = 126.0                           # per-coord quota for regular blocks
    for grp in range(NGRP):
        nodes = np.arange(grp * GRP, (grp + 1) * GRP)
        v = deg[nodes].astype(np.float64)          # [GRP, 16]
        order = np.argsort(-v.sum(axis=1), kind="stable")
        nodes = nodes[order]
        v = v[order]
        ncap = np.full(NB_G, 128, np.int64)
        ncap[NA] = ncap[NA + 1] = SHARD - (NBLK - 1) * 128  # ragged pair
        S = np.zeros((NB_G, NREL * NGRP))
        fill = np.zeros(NB_G, np.int64)
        assign = np.empty(GRP, np.int64)
        # phase 1: heaviest nodes snake-dealt into absorber blocks 0..NA-1
        # (fill only 3/4; the rest of absorber capacity is the relief valve
        # for phase-2 nodes that fit no regular block)
        nheavy = NA * 96
        for j in range(nheavy):
            k = j % (2 * NA)
            bsel = k if k < NA else 2 * NA - 1 - k
            assign[j] = bsel
            S[bsel] += v[j]
            fill[bsel] += 1
        # phase 2: folded order (heavy/light interleaved) + worst-fit:
        # every regular block grows evenly in node count and coord mass.
        K = 32
        rest = np.arange(nheavy, GRP)
        folded = np.empty_like(rest)
        folded[0::2] = rest[: (len(rest) + 1) // 2]
        folded[1::2] = rest[(len(rest) + 1) // 2:][::-1]
        cand_all = NA + 2 + rng_bal.integers(0, NB_G - NA - 2,
                                             size=(GRP, K))
        for j in folded:
            cands = cand_all[j]
            room = fill[cands] < ncap[cands]
            if not room.any():
                cands = np.where(fill < ncap)[0]
                room = np.ones(len(cands), bool)
            Sv = S[cands] + v[j]
            scq = np.where(room, Sv.max(axis=1), np.inf)
            bsel = int(cands[int(np.argmin(scq))])
            assign[j] = bsel
            S[bsel] += v[j]
            fill[bsel] += 1
        # layout: per half (core): regular blocks first, ragged at slot
        # NBLK-1-NA, absorbers at the last NA slots -- aligned across cores.
        reg = [k for k in range(NA + 2, NB_G)]
        key = np.lexsort(S[reg].T)
        reg = [reg[k] for k in key]
        halves = [[], []]
        for k, bk in enumerate(reg):
            halves[k % 2].append(bk)
        for half in range(2):
            # 92 regular + 1 ragged + 5 absorbers; absorber slots align
            # across all cores (same positions), so their 2-chunk cells max
            # together instead of polluting regular slots.
            blocks = halves[half] + [NA + half]
            blocks += [a for a in range(NA) if a % 2 == half]
            cc = 2 * grp + half
            flat = []
            for bk in blocks:
                mem = nodes[assign == bk]
                flat.extend(mem.tolist())
            assert len(flat) == SHARD, (cc, len(flat), len(blocks))
            perm[cc * SHARD: (cc + 1) * SHARD] = np.array(flat, np.int64)
    inv = np.empty(N, np.int64)
    inv[perm] = np.arange(N)

    # relabel edge endpoints into slot space; src groups unchanged by
    # construction (perm permutes within each GRP range)
    src = inv[src]
    dst = inv[dst]
    # cells keyed (block, relation, group); count per core
    NC_CELL = NBLK * NREL * NGRP
    cnt = np.zeros((CORES, NC_CELL), np.int64)
    cell_data = [[None] * NC_CELL for _ in range(CORES)]
    for c in range(CORES):
        for r in range(NREL):
            local = dst[r] - c * SHARD
            m = (local >= 0) & (local < SHARD)
            es = np.nonzero(m)[0]
            loc = local[es]
            b_arr = loc // 128
            g_arr = src[r][es] // GRP
            key = (b_arr * NREL + r) * NGRP + g_arr
            order = np.argsort(key, kind="stable")
            es, loc, key = es[order], loc[order], key[order]
            bounds = np.searchsorted(key, np.arange(NC_CELL + 1))
            si_all = (src[r][es] % GRP).astype(np.int16)
            dl_all = (loc % 128).astype(np.int16)
            wv_all = w_all[r][es].astype(np.float32)
            for b in range(NBLK):
                for g in range(NGRP):
                    cell = (b * NREL + r) * NGRP + g
                    s0, s1 = bounds[cell], bounds[cell + 1]
                    if s0 == s1:
                        continue
                    cell_data[c][cell] = (si_all[s0:s1], dl_all[s0:s1],
                                          wv_all[s0:s1])
                    cnt[c, cell] = s1 - s0

    nch = -(-cnt.max(axis=0) // 128)          # [NC_CELL] chunks per cell

    # call layout: per superblock, per group, chunks from cells
    # (b in sb, r in 0..3); split into calls of <= KMAX chunks.
    sbs = []
    q = 0       # global chunk counter (DSL/WC columns)
    col = 0     # IDX int16 column counter
    for s0 in range(0, NBLK, SBS):
        blocks = list(range(s0, min(s0 + SBS, NBLK)))
        calls = []
        # first/last bookkeeping per (b, r) across the sb
        br_chunks = {}
        for g in range(NGRP):
            descs = []
            tail = []
            for b in blocks:
                for r in range(NREL):
                    cell = (b * NREL + r) * NGRP + g
                    for k in range(int(nch[cell])):
                        if k == 0:
                            descs.append((b, r, g, k, cell))
                        else:
                            tail.append((b, r, g, k, cell))
            # overflow chunks last (likely empty on most cores -> their
            # trailing -1 idxs are trimmed by the Q7 gather at runtime);
            # order by descending mean count so emptier chunks go later
            tail.sort(key=lambda d: -int(cnt[:, d[4]].sum()))
            descs += tail
            for i0 in range(0, len(descs), KMAX):
                part = descs[i0:i0 + KMAX]
                call = dict(g=g, kc=len(part), q0=q, col0=col, part=part)
                calls.append(call)
                for i, (b, r, g_, k, cell) in enumerate(part):
                    br_chunks.setdefault((b, r), []).append(
                        (len(calls) - 1, i))
                q += len(part)
                col += len(part) * 8
        # flags: stop on the last matmul targeting block b within the sb
        b_last = {}
        for ci, call in enumerate(calls):
            for i, (b, r, g_, k, cell) in enumerate(call["part"]):
                b_last[b] = (ci, i)
        for ci, call in enumerate(calls):
            mms = []
            for i, (b, r, g_, k, cell) in enumerate(call["part"]):
                last = b_last.get(b) == (ci, i)
                mms.append((i, b, r, False, last))
            call["mms"] = mms
        empty = [(b, r) for b in blocks for r in range(NREL)
                 if (b, r) not in br_chunks]
        sbs.append(dict(blocks=blocks, calls=calls, empty=empty))
    NCH = q
    NIDXCOL = col

    # chunk data arrays
    IDX = np.zeros((CORES, 16, NIDXCOL), np.int16)
    DSL = np.full((CORES, 128, NCH), -1.0, np.float32)
    WC = np.zeros((CORES, 128, NCH), np.float32)
    for c in range(CORES):
        for sb in sbs:
            for call in sb["calls"]:
                kc = call["kc"]
                last_real = -1  # last idx slot (within call) holding a real edge
                for i, (b, r, g, k, cell) in enumerate(call["part"]):
                    data = cell_data[c][cell]
                    if data is None:
                        continue
                    si, dl, wv = data
                    v0, v1 = k * 128, min((k + 1) * 128, len(si))
                    take = v1 - v0
                    if take <= 0:
                        continue
                    qq = call["q0"] + i
                    nn = i * 128 + np.arange(take)
                    IDX[c, nn % 16, call["col0"] + nn // 16] = si[v0:v1]
                    DSL[c, :take, qq] = dl[v0:v1]
                    WC[c, :take, qq] = wv[v0:v1]
                    last_real = max(last_real, i * 128 + take - 1)
                # (trailing -1 trim disabled: caused device crash)
                del last_real

    # pooling matrix M[slot, r*G + g] = sum_{e in r: src-slot, gid(dst)=g} w/c_g
    # (src/dst already in slot space; gid must be looked up via perm)
    cnt_g = np.maximum(np.bincount(gid, minlength=G), 1.0)
    M = np.zeros((N, MC), np.float32)
    for r in range(NREL):
        gd = gid[perm[dst[r]]]
        np.add.at(M, (src[r], r * G + gd), w_all[r] / cnt_g[gd])
    Mpad = np.zeros((CORES, NBLK * 128, MC), np.float32)
    for c in range(CORES):
        Mpad[c, :SHARD] = M[c * SHARD:(c + 1) * SHARD]

    return dict(sbs=sbs, NCH=NCH, NIDXCOL=NIDXCOL,
                IDX=np.tile(IDX, (1, 8, 1)),
                DSL=DSL.astype(bfloat16), WC=WC.astype(bfloat16),
                M=Mpad.astype(bfloat16), perm=perm)


# ---------------------------------------------------------------------------
# device program
# ---------------------------------------------------------------------------

def _build(plan):
    sbs = plan["sbs"]
    NCH = plan["NCH"]
    NIDXCOL = plan["NIDXCOL"]

    nc = bacc.Bacc(None, target_bir_lowering=False, num_devices=CORES,
                   num_swdge_queues=4)

    p = {}
    p["xTs"] = nc.declare_dram_parameter("xTs", [IN + 1, SHARD], BF16, isOutput=False)
    p["W65"] = nc.declare_dram_parameter("W65", [IN + 1, H], BF16, isOutput=False)
    p["Wl1"] = nc.declare_dram_parameter("Wl1", [NREL, H, H], BF16, isOutput=False)
    p["Wl2"] = nc.declare_dram_parameter("Wl2", [NREL, H, H], BF16, isOutput=False)
    p["Wm1"] = nc.declare_dram_parameter("Wm1", [H, H], BF16, isOutput=False)
    p["Wm2"] = nc.declare_dram_parameter("Wm2", [H, H], BF16, isOutput=False)
    p["Wm3"] = nc.declare_dram_parameter("Wm3", [H, C], BF16, isOutput=False)
    p["B1"] = nc.declare_dram_parameter("B1", [H, 1], F32, isOutput=False)
    p["B2"] = nc.declare_dram_parameter("B2", [H, 1], F32, isOutput=False)
    p["bm1"] = nc.declare_dram_parameter("bm1", [H, 1], F32, isOutput=False)
    p["bm2"] = nc.declare_dram_parameter("bm2", [H, 1], F32, isOutput=False)
    p["bm3"] = nc.declare_dram_parameter("bm3", [C, 1], F32, isOutput=False)
    p["IDX"] = nc.declare_dram_parameter("IDX", [128, NIDXCOL], I16, isOutput=False)
    p["DSL"] = nc.declare_dram_parameter("DSL", [128, NCH], BF16, isOutput=False)
    p["WC"] = nc.declare_dram_parameter("WC", [128, NCH], BF16, isOutput=False)
    p["M"] = nc.declare_dram_parameter("M", [NBLK * 128, MC], BF16, isOutput=False)
    out_ext = nc.declare_dram_parameter("out", [C, G], F32, isOutput=True)

    h0_shard = nc.dram_tensor("h0_shard", [SHARD, H], BF16)
    h0_full = nc.dram_tensor("h0_full", [N, H], BF16, addr_space="Shared")
    pool_in = nc.dram_tensor("pool_in", [H, MC], F32)
    pool_out = nc.dram_tensor("pool_out", [H, MC], F32, addr_space="Shared")

    rg = [list(range(CORES))]

    with tile.TileContext(nc) as tc:
        with (
            tc.tile_pool(name="const", bufs=1) as cpool,
            tc.tile_pool(name="idx", bufs=12) as ipool,
            tc.tile_pool(name="meta", bufs=12) as mpool,
            tc.tile_pool(name="gath", bufs=12) as dpool,
            tc.tile_pool(name="oneh", bufs=10) as opool,
            tc.tile_pool(name="aggs", bufs=2) as apool,
            tc.tile_pool(name="work", bufs=4) as wpool,
            tc.tile_pool(name="pa", bufs=1, space="PSUM") as pa,
            tc.tile_pool(name="po", bufs=2, space="PSUM") as po,
            tc.tile_pool(name="pb", bufs=1, space="PSUM") as pb,
            tc.tile_pool(name="pp", bufs=1, space="PSUM") as pp,
        ):
            # ---- constants
            id_f32 = cpool.tile([128, 128], F32)
            make_identity(nc, id_f32[:])
            id_bf = cpool.tile([128, 128], BF16)
            nc.vector.tensor_copy(id_bf[:], id_f32[:])

            zl = cpool.tile([128, 128], BF16, tag="zl")
            nc.vector.memset(zl[:], 0.0)
            zr = cpool.tile([128, NREL * 128], BF16, tag="zr")
            nc.vector.memset(zr[:], 0.0)

            iota_i = cpool.tile([128, 128], I32)
            nc.gpsimd.iota(iota_i[:], pattern=[[1, 128]], base=0,
                           channel_multiplier=0)
            iota_f = cpool.tile([128, 128], BF16)
            nc.vector.tensor_copy(iota_f[:], iota_i[:])
            # iotaT[p, j, c] = j, materialized so both is_equal operands are
            # inner-contiguous (2 elem/cycle on DVE vs 1 with stride-0 inner)
            iota_t = cpool.tile([128, 128, KMAX], BF16)
            nc.vector.tensor_copy(
                iota_t[:, :, :],
                iota_f[:, :].rearrange("p (f o) -> p f o", o=1)
                .broadcast_to([128, 128, KMAX]))

            w65 = cpool.tile([IN + 1, H], BF16)
            nc.sync.dma_start(w65[:], p["W65"][:, :])
            wl = {}
            for li, name in ((1, "Wl1"), (2, "Wl2")):
                for r in range(NREL):
                    t = cpool.tile([H, H], BF16, tag=f"wl{li}{r}")
                    nc.sync.dma_start(t[:], p[name][r, :, :])
                    wl[(li, r)] = t
            wm = {}
            for name in ("Wm1", "Wm2"):
                t = cpool.tile([H, H], BF16, tag=name)
                nc.sync.dma_start(t[:], p[name][:, :])
                wm[name] = t
            wm3 = cpool.tile([H, C], BF16)
            nc.sync.dma_start(wm3[:], p["Wm3"][:, :])
            biases = {}
            for name in ("B1", "B2", "bm1", "bm2"):
                t = cpool.tile([H, 1], F32, tag=name)
                nc.sync.dma_start(t[:], p[name][:, :])
                biases[name] = t
            bm3 = cpool.tile([C, 1], F32)
            nc.sync.dma_start(bm3[:], p["bm3"][:, :])

            # ---- phase 0: h0 = relu(x @ W_in + b_in) for this core's shard
            with tc.tile_pool(name="ph0", bufs=1) as hpool:
                xs = hpool.tile([IN + 1, SHARD], BF16)
                nc.sync.dma_start(xs[:], p["xTs"][:, :])
                for t in range(NBLK):
                    t0 = t * 128
                    tw = min(128, SHARD - t0)
                    ps = po.tile([128, H], F32, tag="mm")
                    nc.tensor.matmul(ps[:tw, :], lhsT=xs[:, t0:t0 + tw],
                                     rhs=w65[:], start=True, stop=True)
                    hb = wpool.tile([128, H], BF16, tag="h0out")
                    if t % 2 == 0:
                        nc.scalar.activation(hb[:tw, :], ps[:tw, :],
                                             mybir.ActivationFunctionType.Relu)
                    else:
                        nc.vector.tensor_scalar_max(hb[:tw, :], ps[:tw, :], 0.0)
                    nc.sync.dma_start(h0_shard[t0:t0 + tw, :], hb[:tw, :])

            nc.gpsimd.collective_compute(
                "AllGather", mybir.AluOpType.bypass, replica_groups=rg,
                ins=[h0_shard[:, :]], outs=[h0_full[:, :]])

            tables = [h0_full[g * GRP:(g + 1) * GRP, :] for g in range(NGRP)]
            pooled = pp.tile([H, MC], F32, tag="pooled")

            # ---- conv1 + fused pooling
            qi = 0  # running call index for queue round-robin / engine split
            for sb in sbs:
                agg = {}
                for call in sb["calls"]:
                    g, kc, q0, col0 = call["g"], call["kc"], call["q0"], call["col0"]
                    it = ipool.tile([128, KMAX * 8], I16, tag="it")
                    nc.sync.dma_start(it[:, :kc * 8],
                                      p["IDX"][:, col0:col0 + kc * 8])
                    st = dpool.tile([128, KMAX, H], BF16, tag="st")
                    nc.gpsimd.dma_gather(
                        st[:, :kc, :], tables[g], it[:, :kc * 8],
                        kc * 128, kc * 128, H, single_packet=False,
                        queue_num=qi % 4)
                    dslt = mpool.tile([128, KMAX], BF16, tag="dsl")
                    nc.scalar.dma_start(dslt[:, :kc], p["DSL"][:, q0:q0 + kc])
                    wct = mpool.tile([128, KMAX], BF16, tag="wc")
                    nc.scalar.dma_start(wct[:, :kc], p["WC"][:, q0:q0 + kc])
                    # one-hot (slot-major layout): oh[p, j, c] built with
                    # inner-contiguous operands, consumed as strided mm rhs
                    oh = opool.tile([128, 128, KMAX], BF16, tag="oh")
                    dv = dslt[:, :kc].rearrange("p (c o) -> p o c", o=1)\
                        .broadcast_to([128, 128, kc])
                    wv = wct[:, :kc].rearrange("p (c o) -> p o c", o=1)\
                        .broadcast_to([128, 128, kc])
                    nc.vector.tensor_tensor(
                        out=oh[:, :, :kc], in0=dv, in1=iota_t[:, :, :kc],
                        op=mybir.AluOpType.is_equal)
                    nc.vector.tensor_tensor(
                        out=oh[:, :, :kc], in0=oh[:, :, :kc],
                        in1=wv, op=mybir.AluOpType.mult)
                    for (i, b, r, first, last) in call["mms"]:
                        if b not in agg:
                            agg[b] = pa.tile([128, NREL * 128], F32,
                                             name=f"agg{b % SBS}",
                                             tag=f"agg{b % SBS}")
                            # start=True zeroes the whole PSUM bank, so do it
                            # once with a full-width zero matmul; the real
                            # slice matmuls then all accumulate.
                            nc.tensor.matmul(agg[b][:], lhsT=zl[:], rhs=zr[:],
                                             start=True, stop=False)
                        nc.tensor.matmul(agg[b][:, r * 128:(r + 1) * 128],
                                         lhsT=st[:, i, :], rhs=oh[:, :, i],
                                         start=False, stop=last)
                    qi += 1
                # superblock epilogue
                for b in sb["blocks"]:
                    rows = min(128, SHARD - b * 128)
                    ag4 = apool.tile([128, NREL * 128], BF16, tag=f"as{b % 2}")
                    if b in agg:
                        nc.scalar.activation(
                            ag4[:], agg[b][:],
                            mybir.ActivationFunctionType.Copy)
                    else:
                        nc.vector.memset(ag4[:], 0.0)
                    for (eb, er) in sb["empty"]:
                        if eb == b and (b in agg):
                            nc.vector.memset(
                                ag4[:, er * 128:(er + 1) * 128], 0.0)
                    out2 = po.tile([128, 128], F32, tag="mm")
                    for r in range(NREL):
                        nc.tensor.matmul(
                            out2[:], lhsT=wl[(1, r)][:],
                            rhs=ag4[:, r * 128:(r + 1) * 128],
                            start=(r == 0), stop=(r == NREL - 1))
                    t1 = wpool.tile([128, 128], BF16, tag="t1")
                    nc.scalar.activation(
                        t1[:], out2[:],
                        mybir.ActivationFunctionType.Relu,
                        bias=biases["B1"][:, :])
                    tb_ = pb.tile([128, 128], BF16, tag="tb")
                    nc.tensor.transpose(tb_[:], t1[:], id_bf[:])
                    t2 = wpool.tile([128, 128], BF16, tag="t2")
                    nc.scalar.activation(
                        t2[:], tb_[:],
                        mybir.ActivationFunctionType.Copy)
                    mb = wpool.tile([128, MC], BF16, tag="mb")
                    nc.sync.dma_start(
                        mb[:rows, :], p["M"][b * 128:b * 128 + rows, :])
                    nc.tensor.matmul(pooled[:], lhsT=t2[:rows, :],
                                     rhs=mb[:rows, :],
                                     start=(b == 0), stop=(b == NBLK - 1))

            # ---- pooled partial sums -> AllReduce -> W2 contraction -> head
            psb = wpool.tile([H, MC], F32, tag="psb")
            nc.scalar.activation(psb[:], pooled[:],
                                 mybir.ActivationFunctionType.Copy)
            nc.sync.dma_start(pool_in[:, :], psb[:])
            nc.gpsimd.collective_compute(
                "AllReduce", mybir.AluOpType.add, replica_groups=rg,
                ins=[pool_in[:, :]], outs=[pool_out[:, :]])
            pool_f32 = wpool.tile([H, MC], F32, tag="pool_f32")
            nc.sync.dma_start(pool_f32[:], pool_out[:, :])
            pool_f = wpool.tile([H, MC], BF16, tag="pool_f")
            nc.vector.tensor_copy(pool_f[:], pool_f32[:])

            z0p = po.tile([H, G], F32, tag="mm")
            for r in range(NREL):
                nc.tensor.matmul(z0p[:], lhsT=wl[(2, r)][:],
                                 rhs=pool_f[:, r * G:(r + 1) * G],
                                 start=(r == 0), stop=(r == NREL - 1))
            # + B2 (conv2 bias, linear through the mean), cast to bf16
            zb = wpool.tile([H, G], BF16, tag="zb")
            nc.vector.tensor_scalar_add(zb[:], z0p[:], biases["B2"][:, :])

            z1p = po.tile([H, G], F32, tag="mm")
            nc.tensor.matmul(z1p[:], lhsT=wm["Wm1"][:], rhs=zb[:],
                             start=True, stop=True)
            z1 = wpool.tile([H, G], BF16, tag="z1")
            nc.scalar.activation(z1[:], z1p[:],
                                 mybir.ActivationFunctionType.Relu,
                                 bias=biases["bm1"][:, :])
            z2p = po.tile([H, G], F32, tag="mm")
            nc.tensor.matmul(z2p[:], lhsT=wm["Wm2"][:], rhs=z1[:],
                             start=True, stop=True)
            z2 = wpool.tile([H, G], BF16, tag="z2")
            nc.scalar.activation(z2[:], z2p[:],
                                 mybir.ActivationFunctionType.Relu,
                                 bias=biases["bm2"][:, :])
            z3p = po.tile([C, G], F32, tag="mm")
            nc.tensor.matmul(z3p[:], lhsT=wm3[:], rhs=z2[:],
                             start=True, stop=True)
            z3 = wpool.tile([C, G], F32, tag="z3")
            nc.vector.tensor_scalar_add(z3[:], z3p[:], bm3[:, :])
            nc.sync.dma_start(out_ext[:, :], z3[:])

    nc.compile()
    return nc


# ---------------------------------------------------------------------------
# entry point
# ---------------------------------------------------------------------------

_CACHE = {}


def kernel(x, src, dst, graph_ids, W_in, b_in, W1, b1, W2, b2,
           Wm1, bm1, Wm2, bm2, Wm3, bm3):
    x = np.asarray(x)
    key = (int(np.asarray(src).sum()) ^ int(np.asarray(dst).sum()),
           int(np.asarray(graph_ids).sum()))
    if key not in _CACHE:
        plan = _plan(src, dst, graph_ids)
        nc = _build(plan)
        _CACHE[key] = (plan, nc)
    plan, nc = _CACHE[key]

    xT = np.concatenate([np.asarray(x).T, np.ones((1, N), np.float32)], axis=0)
    xT = xT[:, plan["perm"]]
    w65 = np.concatenate([np.asarray(W_in), np.asarray(b_in)[None, :]], axis=0)

    def bf(a):
        return np.ascontiguousarray(np.asarray(a), dtype=np.float32).astype(bfloat16)

    def col(a):
        return np.ascontiguousarray(np.asarray(a, np.float32).reshape(-1, 1))

    xTb = bf(xT)
    in_maps = []
    for c in range(CORES):
        in_maps.append({
            "xTs": np.ascontiguousarray(xTb[:, c * SHARD:(c + 1) * SHARD]),
            "W65": bf(w65),
            "Wl1": bf(W1),
            "Wl2": bf(W2),
            "Wm1": bf(Wm1),
            "Wm2": bf(Wm2),
            "Wm3": bf(Wm3),
            "B1": col(np.asarray(b1, np.float32).sum(axis=0)),
            "B2": col(np.asarray(b2, np.float32).sum(axis=0)),
            "bm1": col(bm1),
            "bm2": col(bm2),
            "bm3": col(bm3),
            "IDX": np.ascontiguousarray(plan["IDX"][c]),
            "DSL": np.ascontiguousarray(plan["DSL"][c]),
            "WC": np.ascontiguousarray(plan["WC"][c]),
            "M": np.ascontiguousarray(plan["M"][c]),
        })

    res = bass_utils.run_bass_kernel_spmd(nc, in_maps, list(range(CORES)))
    global LAST_EXEC_NS
    LAST_EXEC_NS = res.exec_time_ns
    out = np.asarray(res.results[0]["out"], np.float32)  # [C, G]
    return np.ascontiguousarray(out.T)                   # [G, C]


LAST_EXEC_NS = None


if __name__ == "__main__":
    import reference
    import jax
    with jax.default_device(jax.devices("cpu")[0]):
        inp = {k: np.asarray(v) for k, v in reference.setup_inputs().items()}
        exp = np.asarray(reference.reference(**{k: v for k, v in inp.items()}))
    act = kernel(**inp)
    rel = np.linalg.norm(act - exp) / np.linalg.norm(exp)
    print("Relative error:", rel)


# revision 20
# speedup vs baseline: 3.3414x; 1.0773x over previous
"""RGCN (2x hetero GraphConv + mean-pool + MLP) on 8 TRN2 NeuronCores — v3.

Key structure (vs v2 baseline):
- Layer 2 + mean-pooling are algebraically fused into a dense matmul: with no
  relu after conv2 and pooling linear, pooled[g] only needs
  M[n,(r,g)] = sum_{e in r: src=n, gid(dst)=g} w_e / c_g (host-built graph
  metadata). Per dst block: pool += h1_block^T @ M_block. This removes the
  second gather/scatter pass and the h1 AllGather entirely.
- Layer 1 keeps the dst-sharded gather/one-hot-scatter design but with
  relation-pure 128-edge chunks (cells keyed (block, relation, src-group)) so
  one-hots are 128 wide (4x less PE/DVE work than the 512-wide merged form).
- dma_gather calls round-robin across 4 SWDGE queues; each queue maps to a
  different GpSimd Q7 cpu pair, so descriptor generation (the old wall at
  ~9.3 ns/row) runs 4-wide (~2.1 ns/row).
- Per-edge normalization w_e = rsqrt(deg_out_r[src])*rsqrt(deg_in_r[dst]) is a
  tensor_tensor mult of the gathered rows by a broadcast weight column;
  split between DVE and GpSimd to balance engine load.

SPMD: identical instruction stream on all 8 cores; per-core variation lives in
IDX/DSLWC/M tensors. Chunk counts per cell are the max over cores.
"""

import numpy as np
from ml_dtypes import bfloat16

import concourse.bass as bass
import concourse.bacc as bacc
import concourse.mybir as mybir
import concourse.tile as tile
from concourse import bass_utils
from concourse.masks import make_identity

F32 = mybir.dt.float32
BF16 = mybir.dt.bfloat16
I16 = mybir.dt.int16
I32 = mybir.dt.int32

# problem constants (hardcoded per spec)
N, E, NREL, G, IN, H, C = 100000, 400000, 4, 64, 64, 128, 2
CORES = 8
SHARD = N // CORES            # 12500
NBLK = (SHARD + 127) // 128   # 98
GRP = 25000                   # src rows per gather group (int16 idx limit)
NGRP = N // GRP               # 4
SBS = 4                       # dst blocks per superblock (PSUM banks)
KMAX = 16                     # max 128-edge chunks per gather call
MC = NREL * G                 # pooling matrix columns: 256


# ---------------------------------------------------------------------------
# host-side planning: pure graph-structure metadata (indices, degrees, layout)
# ---------------------------------------------------------------------------

def _plan(src, dst, graph_ids):
    src = np.asarray(src).astype(np.int64)
    dst = np.asarray(dst).astype(np.int64)
    gid = np.asarray(graph_ids).astype(np.int64)

    # folded normalization: w_e = rsqrt(deg_in[dst]) * rsqrt(deg_out[src])
    w_all = np.empty((NREL, E), np.float32)
    for r in range(NREL):
        do = np.maximum(np.bincount(src[r], minlength=N), 1.0)
        di = np.maximum(np.bincount(dst[r], minlength=N), 1.0)
        w_all[r] = (1.0 / np.sqrt(do[src[r]]) / np.sqrt(di[dst[r]])).astype(np.float32)

    # ---- node relabeling: permute nodes within each src-group so that the
    # per-(block, relation, group) in-degree is balanced across blocks and
    # cores (pushes chunks/cell toward 1 and minimizes gather padding).
    # perm[slot] = original node id at relabeled slot; inv[n] = slot of n.
    deg = np.zeros((N, NREL * NGRP), np.int32)
    for r in range(NREL):
        gs = src[r] // GRP
        np.add.at(deg, (dst[r], r * NGRP + gs), 1)
    perm = np.empty(N, np.int64)
    rng_bal = np.random.default_rng(12345)
    NB_G = 2 * NBLK                      # blocks per group (2 cores)
    NA = 32                              # absorber blocks per group
    QB = 126.0                           # per-coord quota for regular blocks
    for grp in range(NGRP):
        nodes = np.arange(grp * GRP, (grp + 1) * GRP)
        v = deg[nodes].astype(np.float64)          # [GRP, 16]
        order = np.argsort(-v.sum(axis=1), kind="stable")
        nodes = nodes[order]
        v = v[order]
        ncap = np.full(NB_G, 128, np.int64)
        ncap[NA] = ncap[NA + 1] = SHARD - (NBLK - 1) * 128  # ragged pair
        S = np.zeros((NB_G, NREL * NGRP))
        fill = np.zeros(NB_G, np.int64)
        assign = np.empty(GRP, np.int64)
        # phase 1: heaviest nodes snake-dealt into absorber blocks 0..NA-1
        # (fill only 3/4; the rest of absorber capacity is the relief valve
        # for phase-2 nodes that fit no regular block)
        nheavy = NA * 96
        for j in range(nheavy):
            k = j % (2 * NA)
            bsel = k if k < NA else 2 * NA - 1 - k
            assign[j] = bsel
            S[bsel] += v[j]
            fill[bsel] += 1
        # phase 2: folded order (heavy/light interleaved) + worst-fit:
        # every regular block grows evenly in node count and coord mass.
        K = 32
        rest = np.arange(nheavy, GRP)
        folded = np.empty_like(rest)
        folded[0::2] = rest[: (len(rest) + 1) // 2]
        folded[1::2] = rest[(len(rest) + 1) // 2:][::-1]
        cand_all = NA + 2 + rng_bal.integers(0, NB_G - NA - 2,
                                             size=(GRP, K))
        for j in folded:
            cands = cand_all[j]
            room = fill[cands] < ncap[cands]
            if not room.any():
                cands = np.where(fill < ncap)[0]
                room = np.ones(len(cands), bool)
            Sv = S[cands] + v[j]
            scq = np.where(room, Sv.max(axis=1), np.inf)
            bsel = int(cands[int(np.argmin(scq))])
            assign[j] = bsel
            S[bsel] += v[j]
            fill[bsel] += 1
        # layout: per half (core): regular blocks first, ragged at slot
        # NBLK-1-NA, absorbers at the last NA slots -- aligned across cores.
        reg = [k for k in range(NA + 2, NB_G)]
        key = np.lexsort(S[reg].T)
        reg = [reg[k] for k in key]
        halves = [[], []]
        for k, bk in enumerate(reg):
            halves[k % 2].append(bk)
        for half in range(2):
            # 92 regular + 1 ragged + 5 absorbers; absorber slots align
            # across all cores (same positions), so their 2-chunk cells max
            # together instead of polluting regular slots.
            blocks = halves[half] + [NA + half]
            blocks += [a for a in range(NA) if a % 2 == half]
            cc = 2 * grp + half
            flat = []
            for bk in blocks:
                mem = nodes[assign == bk]
                flat.extend(mem.tolist())
            assert len(flat) == SHARD, (cc, len(flat), len(blocks))
            perm[cc * SHARD: (cc + 1) * SHARD] = np.array(flat, np.int64)
    inv = np.empty(N, np.int64)
    inv[perm] = np.arange(N)

    # relabel edge endpoints into slot space; src groups unchanged by
    # construction (perm permutes within each GRP range)
    src = inv[src]
    dst = inv[dst]
    # cells keyed (block, relation, group); count per core
    NC_CELL = NBLK * NREL * NGRP
    cnt = np.zeros((CORES, NC_CELL), np.int64)
    cell_data = [[None] * NC_CELL for _ in range(CORES)]
    for c in range(CORES):
        for r in range(NREL):
            local = dst[r] - c * SHARD
            m = (local >= 0) & (local < SHARD)
            es = np.nonzero(m)[0]
            loc = local[es]
            b_arr = loc // 128
            g_arr = src[r][es] // GRP
            key = (b_arr * NREL + r) * NGRP + g_arr
            order = np.argsort(key, kind="stable")
            es, loc, key = es[order], loc[order], key[order]
            bounds = np.searchsorted(key, np.arange(NC_CELL + 1))
            si_all = (src[r][es] % GRP).astype(np.int16)
            dl_all = (loc % 128).astype(np.int16)
            wv_all = w_all[r][es].astype(np.float32)
            for b in range(NBLK):
                for g in range(NGRP):
                    cell = (b * NREL + r) * NGRP + g
                    s0, s1 = bounds[cell], bounds[cell + 1]
                    if s0 == s1:
                        continue
                    cell_data[c][cell] = (si_all[s0:s1], dl_all[s0:s1],
                                          wv_all[s0:s1])
                    cnt[c, cell] = s1 - s0

    nch = -(-cnt.max(axis=0) // 128)          # [NC_CELL] chunks per cell

    # call layout: per superblock, per group, chunks from cells
    # (b in sb, r in 0..3); split into calls of <= KMAX chunks.
    sbs = []
    q = 0       # global chunk counter (DSL/WC columns)
    col = 0     # IDX int16 column counter
    for s0 in range(0, NBLK, SBS):
        blocks = list(range(s0, min(s0 + SBS, NBLK)))
        calls = []
        # first/last bookkeeping per (b, r) across the sb
        br_chunks = {}
        for g in range(NGRP):
            descs = []
            tail = []
            for b in blocks:
                for r in range(NREL):
                    cell = (b * NREL + r) * NGRP + g
                    for k in range(int(nch[cell])):
                        if k == 0:
                            descs.append((b, r, g, k, cell))
                        else:
                            tail.append((b, r, g, k, cell))
            # overflow chunks last (likely empty on most cores -> their
            # trailing -1 idxs are trimmed by the Q7 gather at runtime);
            # order by descending mean count so emptier chunks go later
            tail.sort(key=lambda d: -int(cnt[:, d[4]].sum()))
            descs += tail
            for i0 in range(0, len(descs), KMAX):
                part = descs[i0:i0 + KMAX]
                call = dict(g=g, kc=len(part), q0=q, col0=col, part=part)
                calls.append(call)
                for i, (b, r, g_, k, cell) in enumerate(part):
                    br_chunks.setdefault((b, r), []).append(
                        (len(calls) - 1, i))
                q += len(part)
                col += len(part) * 8
        # flags: stop on the last matmul targeting block b within the sb
        b_last = {}
        for ci, call in enumerate(calls):
            for i, (b, r, g_, k, cell) in enumerate(call["part"]):
                b_last[b] = (ci, i)
        for ci, call in enumerate(calls):
            mms = []
            for i, (b, r, g_, k, cell) in enumerate(call["part"]):
                last = b_last.get(b) == (ci, i)
                mms.append((i, b, r, False, last))
            call["mms"] = mms
        empty = [(b, r) for b in blocks for r in range(NREL)
                 if (b, r) not in br_chunks]
        sbs.append(dict(blocks=blocks, calls=calls, empty=empty))
    NCH = q
    NIDXCOL = col

    # chunk data arrays
    IDX = np.zeros((CORES, 16, NIDXCOL), np.int16)
    DSL = np.full((CORES, 128, NCH), -1.0, np.float32)
    WC = np.zeros((CORES, 128, NCH), np.float32)
    for c in range(CORES):
        for sb in sbs:
            for call in sb["calls"]:
                kc = call["kc"]
                last_real = -1  # last idx slot (within call) holding a real edge
                for i, (b, r, g, k, cell) in enumerate(call["part"]):
                    data = cell_data[c][cell]
                    if data is None:
                        continue
                    si, dl, wv = data
                    v0, v1 = k * 128, min((k + 1) * 128, len(si))
                    take = v1 - v0
                    if take <= 0:
                        continue
                    qq = call["q0"] + i
                    nn = i * 128 + np.arange(take)
                    IDX[c, nn % 16, call["col0"] + nn // 16] = si[v0:v1]
                    DSL[c, :take, qq] = dl[v0:v1]
                    WC[c, :take, qq] = wv[v0:v1]
                    last_real = max(last_real, i * 128 + take - 1)
                # (trailing -1 trim disabled: caused device crash)
                del last_real

    # pooling matrix M[slot, r*G + g] = sum_{e in r: src-slot, gid(dst)=g} w/c_g
    # (src/dst already in slot space; gid must be looked up via perm)
    cnt_g = np.maximum(np.bincount(gid, minlength=G), 1.0)
    M = np.zeros((N, MC), np.float32)
    for r in range(NREL):
        gd = gid[perm[dst[r]]]
        np.add.at(M, (src[r], r * G + gd), w_all[r] / cnt_g[gd])
    Mpad = np.zeros((CORES, NBLK * 128, MC), np.float32)
    for c in range(CORES):
        Mpad[c, :SHARD] = M[c * SHARD:(c + 1) * SHARD]

    return dict(sbs=sbs, NCH=NCH, NIDXCOL=NIDXCOL,
                IDX=np.tile(IDX, (1, 8, 1)),
                DSL=DSL.astype(bfloat16), WC=WC.astype(bfloat16),
                M=Mpad.astype(bfloat16), perm=perm)


# ---------------------------------------------------------------------------
# device program
# ---------------------------------------------------------------------------

def _build(plan):
    sbs = plan["sbs"]
    NCH = plan["NCH"]
    NIDXCOL = plan["NIDXCOL"]

    nc = bacc.Bacc(None, target_bir_lowering=False, num_devices=CORES,
                   num_swdge_queues=4)

    p = {}
    p["xTs"] = nc.declare_dram_parameter("xTs", [IN + 1, SHARD], BF16, isOutput=False)
    p["W65"] = nc.declare_dram_parameter("W65", [IN + 1, H], BF16, isOutput=False)
    p["Wl1"] = nc.declare_dram_parameter("Wl1", [NREL, H, H], BF16, isOutput=False)
    p["Wl2"] = nc.declare_dram_parameter("Wl2", [NREL, H, H], BF16, isOutput=False)
    p["Wm1"] = nc.declare_dram_parameter("Wm1", [H, H], BF16, isOutput=False)
    p["Wm2"] = nc.declare_dram_parameter("Wm2", [H, H], BF16, isOutput=False)
    p["Wm3"] = nc.declare_dram_parameter("Wm3", [H, C], BF16, isOutput=False)
    p["B1"] = nc.declare_dram_parameter("B1", [H, 1], F32, isOutput=False)
    p["B2"] = nc.declare_dram_parameter("B2", [H, 1], F32, isOutput=False)
    p["bm1"] = nc.declare_dram_parameter("bm1", [H, 1], F32, isOutput=False)
    p["bm2"] = nc.declare_dram_parameter("bm2", [H, 1], F32, isOutput=False)
    p["bm3"] = nc.declare_dram_parameter("bm3", [C, 1], F32, isOutput=False)
    p["IDX"] = nc.declare_dram_parameter("IDX", [128, NIDXCOL], I16, isOutput=False)
    p["DSL"] = nc.declare_dram_parameter("DSL", [128, NCH], BF16, isOutput=False)
    p["WC"] = nc.declare_dram_parameter("WC", [128, NCH], BF16, isOutput=False)
    p["M"] = nc.declare_dram_parameter("M", [NBLK * 128, MC], BF16, isOutput=False)
    out_ext = nc.declare_dram_parameter("out", [C, G], F32, isOutput=True)

    h0_shard = nc.dram_tensor("h0_shard", [SHARD, H], BF16)
    h0_full = nc.dram_tensor("h0_full", [N, H], BF16, addr_space="Shared")
    pool_in = nc.dram_tensor("pool_in", [H, MC], F32)
    pool_out = nc.dram_tensor("pool_out", [H, MC], F32, addr_space="Shared")

    rg = [list(range(CORES))]

    with tile.TileContext(nc) as tc:
        with (
            tc.tile_pool(name="const", bufs=1) as cpool,
            tc.tile_pool(name="idx", bufs=12) as ipool,
            tc.tile_pool(name="meta", bufs=12) as mpool,
            tc.tile_pool(name="gath", bufs=16) as dpool,
            tc.tile_pool(name="oneh", bufs=10) as opool,
            tc.tile_pool(name="aggs", bufs=2) as apool,
            tc.tile_pool(name="work", bufs=4) as wpool,
            tc.tile_pool(name="pa", bufs=1, space="PSUM") as pa,
            tc.tile_pool(name="po", bufs=2, space="PSUM") as po,
            tc.tile_pool(name="pb", bufs=1, space="PSUM") as pb,
            tc.tile_pool(name="pp", bufs=1, space="PSUM") as pp,
        ):
            # ---- constants
            id_f32 = cpool.tile([128, 128], F32)
            make_identity(nc, id_f32[:])
            id_bf = cpool.tile([128, 128], BF16)
            nc.vector.tensor_copy(id_bf[:], id_f32[:])

            zl = cpool.tile([128, 128], BF16, tag="zl")
            nc.vector.memset(zl[:], 0.0)
            zr = cpool.tile([128, NREL * 128], BF16, tag="zr")
            nc.vector.memset(zr[:], 0.0)

            iota_i = cpool.tile([128, 128], I32)
            nc.gpsimd.iota(iota_i[:], pattern=[[1, 128]], base=0,
                           channel_multiplier=0)
            iota_f = cpool.tile([128, 128], BF16)
            nc.vector.tensor_copy(iota_f[:], iota_i[:])
            # iota4[p, j, l] = j, materialized so both is_equal operands are
            # inner-contiguous (2 elem/cycle on DVE vs 1 with stride-0 inner)
            iota4 = cpool.tile([128, 128, 4], BF16)
            nc.vector.tensor_copy(
                iota4[:, :, :],
                iota_f[:, :].rearrange("p (f o) -> p f o", o=1)
                .broadcast_to([128, 128, 4]))

            w65 = cpool.tile([IN + 1, H], BF16)
            nc.sync.dma_start(w65[:], p["W65"][:, :])
            wl = {}
            for li, name in ((1, "Wl1"), (2, "Wl2")):
                for r in range(NREL):
                    t = cpool.tile([H, H], BF16, tag=f"wl{li}{r}")
                    nc.sync.dma_start(t[:], p[name][r, :, :])
                    wl[(li, r)] = t
            wm = {}
            for name in ("Wm1", "Wm2"):
                t = cpool.tile([H, H], BF16, tag=name)
                nc.sync.dma_start(t[:], p[name][:, :])
                wm[name] = t
            wm3 = cpool.tile([H, C], BF16)
            nc.sync.dma_start(wm3[:], p["Wm3"][:, :])
            biases = {}
            for name in ("B1", "B2", "bm1", "bm2"):
                t = cpool.tile([H, 1], F32, tag=name)
                nc.sync.dma_start(t[:], p[name][:, :])
                biases[name] = t
            bm3 = cpool.tile([C, 1], F32)
            nc.sync.dma_start(bm3[:], p["bm3"][:, :])

            # ---- phase 0: h0 = relu(x @ W_in + b_in) for this core's shard
            with tc.tile_pool(name="ph0", bufs=1) as hpool:
                xs = hpool.tile([IN + 1, SHARD], BF16)
                nc.sync.dma_start(xs[:], p["xTs"][:, :])
                for t in range(NBLK):
                    t0 = t * 128
                    tw = min(128, SHARD - t0)
                    ps = po.tile([128, H], F32, tag="mm")
                    nc.tensor.matmul(ps[:tw, :], lhsT=xs[:, t0:t0 + tw],
                                     rhs=w65[:], start=True, stop=True)
                    hb = wpool.tile([128, H], BF16, tag="h0out")
                    if t % 2 == 0:
                        nc.scalar.activation(hb[:tw, :], ps[:tw, :],
                                             mybir.ActivationFunctionType.Relu)
                    else:
                        nc.vector.tensor_scalar_max(hb[:tw, :], ps[:tw, :], 0.0)
                    nc.sync.dma_start(h0_shard[t0:t0 + tw, :], hb[:tw, :])

            nc.gpsimd.collective_compute(
                "AllGather", mybir.AluOpType.bypass, replica_groups=rg,
                ins=[h0_shard[:, :]], outs=[h0_full[:, :]])

            tables = [h0_full[g * GRP:(g + 1) * GRP, :] for g in range(NGRP)]
            pooled = pp.tile([H, MC], F32, tag="pooled")

            # ---- conv1 + fused pooling
            qi = 0  # running call index for queue round-robin / engine split
            for sb in sbs:
                agg = {}
                for call in sb["calls"]:
                    g, kc, q0, col0 = call["g"], call["kc"], call["q0"], call["col0"]
                    it = ipool.tile([128, KMAX * 8], I16, tag="it")
                    nc.sync.dma_start(it[:, :kc * 8],
                                      p["IDX"][:, col0:col0 + kc * 8])
                    st = dpool.tile([128, KMAX, H], BF16, tag="st")
                    nc.gpsimd.dma_gather(
                        st[:, :kc, :], tables[g], it[:, :kc * 8],
                        kc * 128, kc * 128, H, single_packet=False,
                        queue_num=qi % 4)
                    dslt = mpool.tile([128, KMAX], BF16, tag="dsl")
                    nc.scalar.dma_start(dslt[:, :kc], p["DSL"][:, q0:q0 + kc])
                    wct = mpool.tile([128, KMAX], BF16, tag="wc")
                    nc.scalar.dma_start(wct[:, :kc], p["WC"][:, q0:q0 + kc])
                    # one-hot [p, quad, j, lane]: lane = chunk%4 is the
                    # inner dim so the matmul rhs slice has 8-byte column
                    # stride (free) while both DVE operands stay contiguous
                    qn = (kc + 3) // 4
                    k4 = 4 * qn
                    oh = opool.tile([128, 4, 128, 4], BF16, tag="oh")
                    dv = dslt[:, :k4].rearrange(
                        "p (q o l) -> p q o l", o=1, l=4)\
                        .broadcast_to([128, qn, 128, 4])
                    wv = wct[:, :k4].rearrange(
                        "p (q o l) -> p q o l", o=1, l=4)\
                        .broadcast_to([128, qn, 128, 4])
                    iv = iota4[:, :, :].rearrange(
                        "p (o j) l -> p o j l", o=1)\
                        .broadcast_to([128, qn, 128, 4])
                    nc.vector.tensor_tensor(
                        out=oh[:, :qn, :, :], in0=dv, in1=iv,
                        op=mybir.AluOpType.is_equal)
                    nc.vector.tensor_tensor(
                        out=oh[:, :qn, :, :], in0=oh[:, :qn, :, :],
                        in1=wv, op=mybir.AluOpType.mult)
                    for (i, b, r, first, last) in call["mms"]:
                        if b not in agg:
                            agg[b] = pa.tile([128, NREL * 128], F32,
                                             name=f"agg{b % SBS}",
                                             tag=f"agg{b % SBS}")
                            # start=True zeroes the whole PSUM bank, so do it
                            # once with a full-width zero matmul; the real
                            # slice matmuls then all accumulate.
                            nc.tensor.matmul(agg[b][:], lhsT=zl[:], rhs=zr[:],
                                             start=True, stop=False)
                        nc.tensor.matmul(agg[b][:, r * 128:(r + 1) * 128],
                                         lhsT=st[:, i, :],
                                         rhs=oh[:, i // 4, :, i % 4],
                                         start=False, stop=last)
                    qi += 1
                # superblock epilogue
                for b in sb["blocks"]:
                    rows = min(128, SHARD - b * 128)
                    ag4 = apool.tile([128, NREL * 128], BF16, tag=f"as{b % 2}")
                    if b in agg:
                        nc.scalar.activation(
                            ag4[:], agg[b][:],
                            mybir.ActivationFunctionType.Copy)
                    else:
                        nc.vector.memset(ag4[:], 0.0)
                    for (eb, er) in sb["empty"]:
                        if eb == b and (b in agg):
                            nc.vector.memset(
                                ag4[:, er * 128:(er + 1) * 128], 0.0)
                    out2 = po.tile([128, 128], F32, tag="mm")
                    for r in range(NREL):
                        nc.tensor.matmul(
                            out2[:], lhsT=wl[(1, r)][:],
                            rhs=ag4[:, r * 128:(r + 1) * 128],
                            start=(r == 0), stop=(r == NREL - 1))
                    t1 = wpool.tile([128, 128], BF16, tag="t1")
                    nc.scalar.activation(
                        t1[:], out2[:],
                        mybir.ActivationFunctionType.Relu,
                        bias=biases["B1"][:, :])
                    tb_ = pb.tile([128, 128], BF16, tag="tb")
                    nc.tensor.transpose(tb_[:], t1[:], id_bf[:])
                    t2 = wpool.tile([128, 128], BF16, tag="t2")
                    nc.scalar.activation(
                        t2[:], tb_[:],
                        mybir.ActivationFunctionType.Copy)
                    mb = wpool.tile([128, MC], BF16, tag="mb")
                    nc.sync.dma_start(
                        mb[:rows, :], p["M"][b * 128:b * 128 + rows, :])
                    nc.tensor.matmul(pooled[:], lhsT=t2[:rows, :],
                                     rhs=mb[:rows, :],
                                     start=(b == 0), stop=(b == NBLK - 1))

            # ---- pooled partial sums -> AllReduce -> W2 contraction -> head
            psb = wpool.tile([H, MC], F32, tag="psb")
            nc.scalar.activation(psb[:], pooled[:],
                                 mybir.ActivationFunctionType.Copy)
            nc.sync.dma_start(pool_in[:, :], psb[:])
            nc.gpsimd.collective_compute(
                "AllReduce", mybir.AluOpType.add, replica_groups=rg,
                ins=[pool_in[:, :]], outs=[pool_out[:, :]])
            pool_f32 = wpool.tile([H, MC], F32, tag="pool_f32")
            nc.sync.dma_start(pool_f32[:], pool_out[:, :])
            pool_f = wpool.tile([H, MC], BF16, tag="pool_f")
            nc.vector.tensor_copy(pool_f[:], pool_f32[:])

            z0p = po.tile([H, G], F32, tag="mm")
            for r in range(NREL):
                nc.tensor.matmul(z0p[:], lhsT=wl[(2, r)][:],
                                 rhs=pool_f[:, r * G:(r + 1) * G],
                                 start=(r == 0), stop=(r == NREL - 1))
            # + B2 (conv2 bias, linear through the mean), cast to bf16
            zb = wpool.tile([H, G], BF16, tag="zb")
            nc.vector.tensor_scalar_add(zb[:], z0p[:], biases["B2"][:, :])

            z1p = po.tile([H, G], F32, tag="mm")
            nc.tensor.matmul(z1p[:], lhsT=wm["Wm1"][:], rhs=zb[:],
                             start=True, stop=True)
            z1 = wpool.tile([H, G], BF16, tag="z1")
            nc.scalar.activation(z1[:], z1p[:],
                                 mybir.ActivationFunctionType.Relu,
                                 bias=biases["bm1"][:, :])
            z2p = po.tile([H, G], F32, tag="mm")
            nc.tensor.matmul(z2p[:], lhsT=wm["Wm2"][:], rhs=z1[:],
                             start=True, stop=True)
            z2 = wpool.tile([H, G], BF16, tag="z2")
            nc.scalar.activation(z2[:], z2p[:],
                                 mybir.ActivationFunctionType.Relu,
                                 bias=biases["bm2"][:, :])
            z3p = po.tile([C, G], F32, tag="mm")
            nc.tensor.matmul(z3p[:], lhsT=wm3[:], rhs=z2[:],
                             start=True, stop=True)
            z3 = wpool.tile([C, G], F32, tag="z3")
            nc.vector.tensor_scalar_add(z3[:], z3p[:], bm3[:, :])
            nc.sync.dma_start(out_ext[:, :], z3[:])

    nc.compile()
    return nc


# ---------------------------------------------------------------------------
# entry point
# ---------------------------------------------------------------------------

_CACHE = {}


def kernel(x, src, dst, graph_ids, W_in, b_in, W1, b1, W2, b2,
           Wm1, bm1, Wm2, bm2, Wm3, bm3):
    x = np.asarray(x)
    key = (int(np.asarray(src).sum()) ^ int(np.asarray(dst).sum()),
           int(np.asarray(graph_ids).sum()))
    if key not in _CACHE:
        plan = _plan(src, dst, graph_ids)
        nc = _build(plan)
        _CACHE[key] = (plan, nc)
    plan, nc = _CACHE[key]

    xT = np.concatenate([np.asarray(x).T, np.ones((1, N), np.float32)], axis=0)
    xT = xT[:, plan["perm"]]
    w65 = np.concatenate([np.asarray(W_in), np.asarray(b_in)[None, :]], axis=0)

    def bf(a):
        return np.ascontiguousarray(np.asarray(a), dtype=np.float32).astype(bfloat16)

    def col(a):
        return np.ascontiguousarray(np.asarray(a, np.float32).reshape(-1, 1))

    xTb = bf(xT)
    in_maps = []
    for c in range(CORES):
        in_maps.append({
            "xTs": np.ascontiguousarray(xTb[:, c * SHARD:(c + 1) * SHARD]),
            "W65": bf(w65),
            "Wl1": bf(W1),
            "Wl2": bf(W2),
            "Wm1": bf(Wm1),
            "Wm2": bf(Wm2),
            "Wm3": bf(Wm3),
            "B1": col(np.asarray(b1, np.float32).sum(axis=0)),
            "B2": col(np.asarray(b2, np.float32).sum(axis=0)),
            "bm1": col(bm1),
            "bm2": col(bm2),
            "bm3": col(bm3),
            "IDX": np.ascontiguousarray(plan["IDX"][c]),
            "DSL": np.ascontiguousarray(plan["DSL"][c]),
            "WC": np.ascontiguousarray(plan["WC"][c]),
            "M": np.ascontiguousarray(plan["M"][c]),
        })

    res = bass_utils.run_bass_kernel_spmd(nc, in_maps, list(range(CORES)))
    global LAST_EXEC_NS
    LAST_EXEC_NS = res.exec_time_ns
    out = np.asarray(res.results[0]["out"], np.float32)  # [C, G]
    return np.ascontiguousarray(out.T)                   # [G, C]


LAST_EXEC_NS = None


if __name__ == "__main__":
    import reference
    import jax
    with jax.default_device(jax.devices("cpu")[0]):
        inp = {k: np.asarray(v) for k, v in reference.setup_inputs().items()}
        exp = np.asarray(reference.reference(**{k: v for k, v in inp.items()}))
    act = kernel(**inp)
    rel = np.linalg.norm(act - exp) / np.linalg.norm(exp)
    print("Relative error:", rel)


# revision 22
# speedup vs baseline: 3.6905x; 1.1045x over previous
"""RGCN (2x hetero GraphConv + mean-pool + MLP) on 8 TRN2 NeuronCores — v3.

Key structure (vs v2 baseline):
- Layer 2 + mean-pooling are algebraically fused into a dense matmul: with no
  relu after conv2 and pooling linear, pooled[g] only needs
  M[n,(r,g)] = sum_{e in r: src=n, gid(dst)=g} w_e / c_g (host-built graph
  metadata). Per dst block: pool += h1_block^T @ M_block. This removes the
  second gather/scatter pass and the h1 AllGather entirely.
- Layer 1 keeps the dst-sharded gather/one-hot-scatter design but with
  relation-pure 128-edge chunks (cells keyed (block, relation, src-group)) so
  one-hots are 128 wide (4x less PE/DVE work than the 512-wide merged form).
- dma_gather calls round-robin across 4 SWDGE queues; each queue maps to a
  different GpSimd Q7 cpu pair, so descriptor generation (the old wall at
  ~9.3 ns/row) runs 4-wide (~2.1 ns/row).
- Per-edge normalization w_e = rsqrt(deg_out_r[src])*rsqrt(deg_in_r[dst]) is a
  tensor_tensor mult of the gathered rows by a broadcast weight column;
  split between DVE and GpSimd to balance engine load.

SPMD: identical instruction stream on all 8 cores; per-core variation lives in
IDX/DSLWC/M tensors. Chunk counts per cell are the max over cores.
"""

import numpy as np
from ml_dtypes import bfloat16

import concourse.bass as bass
import concourse.bacc as bacc
import concourse.mybir as mybir
import concourse.tile as tile
from concourse import bass_utils
from concourse.masks import make_identity

F32 = mybir.dt.float32
BF16 = mybir.dt.bfloat16
I16 = mybir.dt.int16
I32 = mybir.dt.int32

# problem constants (hardcoded per spec)
N, E, NREL, G, IN, H, C = 100000, 400000, 4, 64, 64, 128, 2
CORES = 8
SHARD = N // CORES            # 12500
NBLK = (SHARD + 127) // 128   # 98
GRP = 25000                   # src rows per gather group (int16 idx limit)
NGRP = N // GRP               # 4
SBS = 4                       # dst blocks per superblock (PSUM banks)
KMAX = 16                     # max 128-edge chunks per gather call
MC = NREL * G                 # pooling matrix columns: 256


# ---------------------------------------------------------------------------
# host-side planning: pure graph-structure metadata (indices, degrees, layout)
# ---------------------------------------------------------------------------

def _plan(src, dst, graph_ids):
    src = np.asarray(src).astype(np.int64)
    dst = np.asarray(dst).astype(np.int64)
    gid = np.asarray(graph_ids).astype(np.int64)

    # folded normalization: w_e = rsqrt(deg_in[dst]) * rsqrt(deg_out[src])
    w_all = np.empty((NREL, E), np.float32)
    for r in range(NREL):
        do = np.maximum(np.bincount(src[r], minlength=N), 1.0)
        di = np.maximum(np.bincount(dst[r], minlength=N), 1.0)
        w_all[r] = (1.0 / np.sqrt(do[src[r]]) / np.sqrt(di[dst[r]])).astype(np.float32)

    # ---- node relabeling: permute nodes within each src-group so that the
    # per-(block, relation, group) in-degree is balanced across blocks and
    # cores (pushes chunks/cell toward 1 and minimizes gather padding).
    # perm[slot] = original node id at relabeled slot; inv[n] = slot of n.
    deg = np.zeros((N, NREL * NGRP), np.int32)
    for r in range(NREL):
        gs = src[r] // GRP
        np.add.at(deg, (dst[r], r * NGRP + gs), 1)
    perm = np.empty(N, np.int64)
    rng_bal = np.random.default_rng(12345)
    NB_G = 2 * NBLK                      # blocks per group (2 cores)
    NA = 32                              # absorber blocks per group
    QB = 126.0                           # per-coord quota for regular blocks
    for grp in range(NGRP):
        nodes = np.arange(grp * GRP, (grp + 1) * GRP)
        v = deg[nodes].astype(np.float64)          # [GRP, 16]
        order = np.argsort(-v.sum(axis=1), kind="stable")
        nodes = nodes[order]
        v = v[order]
        ncap = np.full(NB_G, 128, np.int64)
        ncap[NA] = ncap[NA + 1] = SHARD - (NBLK - 1) * 128  # ragged pair
        S = np.zeros((NB_G, NREL * NGRP))
        fill = np.zeros(NB_G, np.int64)
        assign = np.empty(GRP, np.int64)
        # phase 1: heaviest nodes snake-dealt into absorber blocks 0..NA-1
        # (fill only 3/4; the rest of absorber capacity is the relief valve
        # for phase-2 nodes that fit no regular block)
        nheavy = NA * 96
        for j in range(nheavy):
            k = j % (2 * NA)
            bsel = k if k < NA else 2 * NA - 1 - k
            assign[j] = bsel
            S[bsel] += v[j]
            fill[bsel] += 1
        # phase 2: folded order (heavy/light interleaved) + worst-fit:
        # every regular block grows evenly in node count and coord mass.
        K = 32
        rest = np.arange(nheavy, GRP)
        folded = np.empty_like(rest)
        folded[0::2] = rest[: (len(rest) + 1) // 2]
        folded[1::2] = rest[(len(rest) + 1) // 2:][::-1]
        cand_all = NA + 2 + rng_bal.integers(0, NB_G - NA - 2,
                                             size=(GRP, K))
        for j in folded:
            cands = cand_all[j]
            room = fill[cands] < ncap[cands]
            if not room.any():
                cands = np.where(fill < ncap)[0]
                room = np.ones(len(cands), bool)
            Sv = S[cands] + v[j]
            scq = np.where(room, Sv.max(axis=1), np.inf)
            bsel = int(cands[int(np.argmin(scq))])
            assign[j] = bsel
            S[bsel] += v[j]
            fill[bsel] += 1
        # layout: per half (core): regular blocks first, ragged at slot
        # NBLK-1-NA, absorbers at the last NA slots -- aligned across cores.
        reg = [k for k in range(NA + 2, NB_G)]
        key = np.lexsort(S[reg].T)
        reg = [reg[k] for k in key]
        halves = [[], []]
        for k, bk in enumerate(reg):
            halves[k % 2].append(bk)
        for half in range(2):
            # 92 regular + 1 ragged + 5 absorbers; absorber slots align
            # across all cores (same positions), so their 2-chunk cells max
            # together instead of polluting regular slots.
            blocks = halves[half] + [NA + half]
            blocks += [a for a in range(NA) if a % 2 == half]
            cc = 2 * grp + half
            flat = []
            for bk in blocks:
                mem = nodes[assign == bk]
                flat.extend(mem.tolist())
            assert len(flat) == SHARD, (cc, len(flat), len(blocks))
            perm[cc * SHARD: (cc + 1) * SHARD] = np.array(flat, np.int64)
    inv = np.empty(N, np.int64)
    inv[perm] = np.arange(N)

    # relabel edge endpoints into slot space; src groups unchanged by
    # construction (perm permutes within each GRP range)
    src = inv[src]
    dst = inv[dst]
    # cells keyed (block, relation, group); count per core
    NC_CELL = NBLK * NREL * NGRP
    cnt = np.zeros((CORES, NC_CELL), np.int64)
    cell_data = [[None] * NC_CELL for _ in range(CORES)]
    for c in range(CORES):
        for r in range(NREL):
            local = dst[r] - c * SHARD
            m = (local >= 0) & (local < SHARD)
            es = np.nonzero(m)[0]
            loc = local[es]
            b_arr = loc // 128
            g_arr = src[r][es] // GRP
            key = (b_arr * NREL + r) * NGRP + g_arr
            order = np.argsort(key, kind="stable")
            es, loc, key = es[order], loc[order], key[order]
            bounds = np.searchsorted(key, np.arange(NC_CELL + 1))
            si_all = (src[r][es] % GRP).astype(np.int16)
            dl_all = (loc % 128).astype(np.int16)
            wv_all = w_all[r][es].astype(np.float32)
            for b in range(NBLK):
                for g in range(NGRP):
                    cell = (b * NREL + r) * NGRP + g
                    s0, s1 = bounds[cell], bounds[cell + 1]
                    if s0 == s1:
                        continue
                    cell_data[c][cell] = (si_all[s0:s1], dl_all[s0:s1],
                                          wv_all[s0:s1])
                    cnt[c, cell] = s1 - s0

    nch = -(-cnt.max(axis=0) // 128)          # [NC_CELL] chunks per cell

    # call layout: per superblock, per group, chunks from cells
    # (b in sb, r in 0..3); split into calls of <= KMAX chunks.
    sbs = []
    q = 0       # global chunk counter (DSL/WC columns)
    col = 0     # IDX int16 column counter
    for s0 in range(0, NBLK, SBS):
        blocks = list(range(s0, min(s0 + SBS, NBLK)))
        calls = []
        # first/last bookkeeping per (b, r) across the sb
        br_chunks = {}
        for g in range(NGRP):
            descs = []
            tail = []
            for b in blocks:
                for r in range(NREL):
                    cell = (b * NREL + r) * NGRP + g
                    for k in range(int(nch[cell])):
                        if k == 0:
                            descs.append((b, r, g, k, cell))
                        else:
                            tail.append((b, r, g, k, cell))
            # overflow chunks last (likely empty on most cores -> their
            # trailing -1 idxs are trimmed by the Q7 gather at runtime);
            # order by descending mean count so emptier chunks go later
            tail.sort(key=lambda d: -int(cnt[:, d[4]].sum()))
            descs += tail
            for i0 in range(0, len(descs), KMAX):
                part = descs[i0:i0 + KMAX]
                call = dict(g=g, kc=len(part), q0=q, col0=col, part=part)
                calls.append(call)
                for i, (b, r, g_, k, cell) in enumerate(part):
                    br_chunks.setdefault((b, r), []).append(
                        (len(calls) - 1, i))
                q += len(part)
                col += len(part) * 8
        # flags: stop on the last matmul targeting block b within the sb
        b_last = {}
        for ci, call in enumerate(calls):
            for i, (b, r, g_, k, cell) in enumerate(call["part"]):
                b_last[b] = (ci, i)
        for ci, call in enumerate(calls):
            mms = []
            for i, (b, r, g_, k, cell) in enumerate(call["part"]):
                last = b_last.get(b) == (ci, i)
                mms.append((i, b, r, False, last))
            call["mms"] = mms
        empty = [(b, r) for b in blocks for r in range(NREL)
                 if (b, r) not in br_chunks]
        sbs.append(dict(blocks=blocks, calls=calls, empty=empty))
    NCH = q
    NIDXCOL = col

    # chunk data arrays (+3 pad columns so 4-aligned slices never overrun)
    IDX = np.zeros((CORES, 16, NIDXCOL), np.int16)
    DSL = np.full((CORES, 128, NCH + 3), -1.0, np.float32)
    WC = np.zeros((CORES, 128, NCH + 3), np.float32)
    for c in range(CORES):
        for sb in sbs:
            for call in sb["calls"]:
                kc = call["kc"]
                last_real = -1  # last idx slot (within call) holding a real edge
                for i, (b, r, g, k, cell) in enumerate(call["part"]):
                    data = cell_data[c][cell]
                    if data is None:
                        continue
                    si, dl, wv = data
                    v0, v1 = k * 128, min((k + 1) * 128, len(si))
                    take = v1 - v0
                    if take <= 0:
                        continue
                    qq = call["q0"] + i
                    nn = i * 128 + np.arange(take)
                    IDX[c, nn % 16, call["col0"] + nn // 16] = si[v0:v1]
                    DSL[c, :take, qq] = dl[v0:v1]
                    WC[c, :take, qq] = wv[v0:v1]
                    last_real = max(last_real, i * 128 + take - 1)
                # (trailing -1 trim disabled: caused device crash)
                del last_real

    # pooling matrix M[slot, r*G + g] = sum_{e in r: src-slot, gid(dst)=g} w/c_g
    # (src/dst already in slot space; gid must be looked up via perm)
    cnt_g = np.maximum(np.bincount(gid, minlength=G), 1.0)
    M = np.zeros((N, MC), np.float32)
    for r in range(NREL):
        gd = gid[perm[dst[r]]]
        np.add.at(M, (src[r], r * G + gd), w_all[r] / cnt_g[gd])
    Mpad = np.zeros((CORES, NBLK * 128, MC), np.float32)
    for c in range(CORES):
        Mpad[c, :SHARD] = M[c * SHARD:(c + 1) * SHARD]

    return dict(sbs=sbs, NCH=NCH, NIDXCOL=NIDXCOL,
                IDX=np.tile(IDX, (1, 8, 1)),
                DSL=DSL.astype(bfloat16), WC=WC.astype(bfloat16),
                M=Mpad.astype(bfloat16), perm=perm)


# ---------------------------------------------------------------------------
# device program
# ---------------------------------------------------------------------------

def _build(plan):
    sbs = plan["sbs"]
    NCH = plan["NCH"]
    NIDXCOL = plan["NIDXCOL"]

    nc = bacc.Bacc(None, target_bir_lowering=False, num_devices=CORES,
                   num_swdge_queues=4)

    p = {}
    p["xTs"] = nc.declare_dram_parameter("xTs", [IN + 1, SHARD], BF16, isOutput=False)
    p["W65"] = nc.declare_dram_parameter("W65", [IN + 1, H], BF16, isOutput=False)
    p["Wl1"] = nc.declare_dram_parameter("Wl1", [NREL, H, H], BF16, isOutput=False)
    p["Wl2"] = nc.declare_dram_parameter("Wl2", [NREL, H, H], BF16, isOutput=False)
    p["Wm1"] = nc.declare_dram_parameter("Wm1", [H, H], BF16, isOutput=False)
    p["Wm2"] = nc.declare_dram_parameter("Wm2", [H, H], BF16, isOutput=False)
    p["Wm3"] = nc.declare_dram_parameter("Wm3", [H, C], BF16, isOutput=False)
    p["B1"] = nc.declare_dram_parameter("B1", [H, 1], F32, isOutput=False)
    p["B2"] = nc.declare_dram_parameter("B2", [H, 1], F32, isOutput=False)
    p["bm1"] = nc.declare_dram_parameter("bm1", [H, 1], F32, isOutput=False)
    p["bm2"] = nc.declare_dram_parameter("bm2", [H, 1], F32, isOutput=False)
    p["bm3"] = nc.declare_dram_parameter("bm3", [C, 1], F32, isOutput=False)
    p["IDX"] = nc.declare_dram_parameter("IDX", [128, NIDXCOL], I16, isOutput=False)
    p["DSL"] = nc.declare_dram_parameter("DSL", [128, NCH + 3], BF16, isOutput=False)
    p["WC"] = nc.declare_dram_parameter("WC", [128, NCH + 3], BF16, isOutput=False)
    p["M"] = nc.declare_dram_parameter("M", [NBLK * 128, MC], BF16, isOutput=False)
    out_ext = nc.declare_dram_parameter("out", [C, G], F32, isOutput=True)

    h0_shard = nc.dram_tensor("h0_shard", [SHARD, H], BF16)
    h0_full = nc.dram_tensor("h0_full", [N, H], BF16, addr_space="Shared")
    pool_in = nc.dram_tensor("pool_in", [H, MC], F32)
    pool_out = nc.dram_tensor("pool_out", [H, MC], F32, addr_space="Shared")

    rg = [list(range(CORES))]

    with tile.TileContext(nc) as tc:
        with (
            tc.tile_pool(name="const", bufs=1) as cpool,
            tc.tile_pool(name="gath", bufs=16) as dpool,
            tc.tile_pool(name="oneh", bufs=10) as opool,
            tc.tile_pool(name="aggs", bufs=2) as apool,
            tc.tile_pool(name="work", bufs=4) as wpool,
            tc.tile_pool(name="pa", bufs=1, space="PSUM") as pa,
            tc.tile_pool(name="po", bufs=2, space="PSUM") as po,
            tc.tile_pool(name="pb", bufs=1, space="PSUM") as pb,
            tc.tile_pool(name="pp", bufs=1, space="PSUM") as pp,
        ):
            # ---- constants
            id_f32 = cpool.tile([128, 128], F32)
            make_identity(nc, id_f32[:])
            id_bf = cpool.tile([128, 128], BF16)
            nc.vector.tensor_copy(id_bf[:], id_f32[:])

            zl = cpool.tile([128, 128], BF16, tag="zl")
            nc.vector.memset(zl[:], 0.0)
            zr = cpool.tile([128, NREL * 128], BF16, tag="zr")
            nc.vector.memset(zr[:], 0.0)

            iota_i = cpool.tile([128, 128], I32)
            nc.gpsimd.iota(iota_i[:], pattern=[[1, 128]], base=0,
                           channel_multiplier=0)
            iota_f = cpool.tile([128, 128], BF16)
            nc.vector.tensor_copy(iota_f[:], iota_i[:])
            # iota4[p, j, l] = j, materialized so both is_equal operands are
            # inner-contiguous (2 elem/cycle on DVE vs 1 with stride-0 inner)
            iota4 = cpool.tile([128, 128, 4], BF16)
            nc.vector.tensor_copy(
                iota4[:, :, :],
                iota_f[:, :].rearrange("p (f o) -> p f o", o=1)
                .broadcast_to([128, 128, 4]))

            w65 = cpool.tile([IN + 1, H], BF16)
            nc.sync.dma_start(w65[:], p["W65"][:, :])
            wl = {}
            for li, name in ((1, "Wl1"), (2, "Wl2")):
                for r in range(NREL):
                    t = cpool.tile([H, H], BF16, tag=f"wl{li}{r}")
                    nc.sync.dma_start(t[:], p[name][r, :, :])
                    wl[(li, r)] = t
            wm = {}
            for name in ("Wm1", "Wm2"):
                t = cpool.tile([H, H], BF16, tag=name)
                nc.sync.dma_start(t[:], p[name][:, :])
                wm[name] = t
            wm3 = cpool.tile([H, C], BF16)
            nc.sync.dma_start(wm3[:], p["Wm3"][:, :])
            biases = {}
            for name in ("B1", "B2", "bm1", "bm2"):
                t = cpool.tile([H, 1], F32, tag=name)
                nc.sync.dma_start(t[:], p[name][:, :])
                biases[name] = t
            bm3 = cpool.tile([C, 1], F32)
            nc.sync.dma_start(bm3[:], p["bm3"][:, :])

            # whole-array preloads: kills per-call metadata DMAs and the
            # dispatch-blocking semaphore waits in front of every gather
            it_all = cpool.tile([128, NIDXCOL], I16, tag="it_all")
            nc.sync.dma_start(it_all[:], p["IDX"][:, :])
            dsl_all = cpool.tile([128, NCH + 3], BF16, tag="dsl_all")
            nc.scalar.dma_start(dsl_all[:], p["DSL"][:, :])
            wc_all = cpool.tile([128, NCH + 3], BF16, tag="wc_all")
            nc.scalar.dma_start(wc_all[:], p["WC"][:, :])

            # ---- phase 0: h0 = relu(x @ W_in + b_in) for this core's shard
            with tc.tile_pool(name="ph0", bufs=1) as hpool:
                xs = hpool.tile([IN + 1, SHARD], BF16)
                nc.sync.dma_start(xs[:], p["xTs"][:, :])
                for t in range(NBLK):
                    t0 = t * 128
                    tw = min(128, SHARD - t0)
                    ps = po.tile([128, H], F32, tag="mm")
                    nc.tensor.matmul(ps[:tw, :], lhsT=xs[:, t0:t0 + tw],
                                     rhs=w65[:], start=True, stop=True)
                    hb = wpool.tile([128, H], BF16, tag="h0out")
                    if t % 2 == 0:
                        nc.scalar.activation(hb[:tw, :], ps[:tw, :],
                                             mybir.ActivationFunctionType.Relu)
                    else:
                        nc.vector.tensor_scalar_max(hb[:tw, :], ps[:tw, :], 0.0)
                    nc.sync.dma_start(h0_shard[t0:t0 + tw, :], hb[:tw, :])

            nc.gpsimd.collective_compute(
                "AllGather", mybir.AluOpType.bypass, replica_groups=rg,
                ins=[h0_shard[:, :]], outs=[h0_full[:, :]])

            tables = [h0_full[g * GRP:(g + 1) * GRP, :] for g in range(NGRP)]
            pooled = pp.tile([H, MC], F32, tag="pooled")

            # ---- conv1 + fused pooling
            qi = 0  # running call index for queue round-robin / engine split
            for sb in sbs:
                agg = {}
                for call in sb["calls"]:
                    g, kc, q0, col0 = call["g"], call["kc"], call["q0"], call["col0"]
                    st = dpool.tile([128, KMAX, H], BF16, tag="st")
                    nc.gpsimd.dma_gather(
                        st[:, :kc, :], tables[g],
                        it_all[:, col0:col0 + kc * 8],
                        kc * 128, kc * 128, H, single_packet=False,
                        queue_num=qi % 4)
                    # one-hot [p, quad, j, lane]: lane = chunk%4 is the
                    # inner dim so the matmul rhs slice has 8-byte column
                    # stride (free) while both DVE operands stay contiguous
                    qn = (kc + 3) // 4
                    k4 = 4 * qn
                    oh = opool.tile([128, 4, 128, 4], BF16, tag="oh")
                    dv = dsl_all[:, q0:q0 + k4].rearrange(
                        "p (q o l) -> p q o l", o=1, l=4)\
                        .broadcast_to([128, qn, 128, 4])
                    wv = wc_all[:, q0:q0 + k4].rearrange(
                        "p (q o l) -> p q o l", o=1, l=4)\
                        .broadcast_to([128, qn, 128, 4])
                    iv = iota4[:, :, :].rearrange(
                        "p (o j) l -> p o j l", o=1)\
                        .broadcast_to([128, qn, 128, 4])
                    nc.vector.tensor_tensor(
                        out=oh[:, :qn, :, :], in0=dv, in1=iv,
                        op=mybir.AluOpType.is_equal)
                    nc.vector.tensor_tensor(
                        out=oh[:, :qn, :, :], in0=oh[:, :qn, :, :],
                        in1=wv, op=mybir.AluOpType.mult)
                    for (i, b, r, first, last) in call["mms"]:
                        if b not in agg:
                            agg[b] = pa.tile([128, NREL * 128], F32,
                                             name=f"agg{b % SBS}",
                                             tag=f"agg{b % SBS}")
                            # start=True zeroes the whole PSUM bank, so do it
                            # once with a full-width zero matmul; the real
                            # slice matmuls then all accumulate.
                            nc.tensor.matmul(agg[b][:], lhsT=zl[:], rhs=zr[:],
                                             start=True, stop=False)
                        nc.tensor.matmul(agg[b][:, r * 128:(r + 1) * 128],
                                         lhsT=st[:, i, :],
                                         rhs=oh[:, i // 4, :, i % 4],
                                         start=False, stop=last)
                    qi += 1
                # superblock epilogue
                for b in sb["blocks"]:
                    rows = min(128, SHARD - b * 128)
                    ag4 = apool.tile([128, NREL * 128], BF16, tag=f"as{b % 2}")
                    if b in agg:
                        nc.scalar.activation(
                            ag4[:], agg[b][:],
                            mybir.ActivationFunctionType.Copy)
                    else:
                        nc.vector.memset(ag4[:], 0.0)
                    for (eb, er) in sb["empty"]:
                        if eb == b and (b in agg):
                            nc.vector.memset(
                                ag4[:, er * 128:(er + 1) * 128], 0.0)
                    out2 = po.tile([128, 128], F32, tag="mm")
                    for r in range(NREL):
                        nc.tensor.matmul(
                            out2[:], lhsT=wl[(1, r)][:],
                            rhs=ag4[:, r * 128:(r + 1) * 128],
                            start=(r == 0), stop=(r == NREL - 1))
                    t1 = wpool.tile([128, 128], BF16, tag="t1")
                    nc.scalar.activation(
                        t1[:], out2[:],
                        mybir.ActivationFunctionType.Relu,
                        bias=biases["B1"][:, :])
                    tb_ = pb.tile([128, 128], BF16, tag="tb")
                    nc.tensor.transpose(tb_[:], t1[:], id_bf[:])
                    t2 = wpool.tile([128, 128], BF16, tag="t2")
                    nc.scalar.activation(
                        t2[:], tb_[:],
                        mybir.ActivationFunctionType.Copy)
                    mb = wpool.tile([128, MC], BF16, tag="mb")
                    nc.sync.dma_start(
                        mb[:rows, :], p["M"][b * 128:b * 128 + rows, :])
                    nc.tensor.matmul(pooled[:], lhsT=t2[:rows, :],
                                     rhs=mb[:rows, :],
                                     start=(b == 0), stop=(b == NBLK - 1))

            # ---- pooled partial sums -> AllReduce -> W2 contraction -> head
            psb = wpool.tile([H, MC], F32, tag="psb")
            nc.scalar.activation(psb[:], pooled[:],
                                 mybir.ActivationFunctionType.Copy)
            nc.sync.dma_start(pool_in[:, :], psb[:])
            nc.gpsimd.collective_compute(
                "AllReduce", mybir.AluOpType.add, replica_groups=rg,
                ins=[pool_in[:, :]], outs=[pool_out[:, :]])
            pool_f32 = wpool.tile([H, MC], F32, tag="pool_f32")
            nc.sync.dma_start(pool_f32[:], pool_out[:, :])
            pool_f = wpool.tile([H, MC], BF16, tag="pool_f")
            nc.vector.tensor_copy(pool_f[:], pool_f32[:])

            z0p = po.tile([H, G], F32, tag="mm")
            for r in range(NREL):
                nc.tensor.matmul(z0p[:], lhsT=wl[(2, r)][:],
                                 rhs=pool_f[:, r * G:(r + 1) * G],
                                 start=(r == 0), stop=(r == NREL - 1))
            # + B2 (conv2 bias, linear through the mean), cast to bf16
            zb = wpool.tile([H, G], BF16, tag="zb")
            nc.vector.tensor_scalar_add(zb[:], z0p[:], biases["B2"][:, :])

            z1p = po.tile([H, G], F32, tag="mm")
            nc.tensor.matmul(z1p[:], lhsT=wm["Wm1"][:], rhs=zb[:],
                             start=True, stop=True)
            z1 = wpool.tile([H, G], BF16, tag="z1")
            nc.scalar.activation(z1[:], z1p[:],
                                 mybir.ActivationFunctionType.Relu,
                                 bias=biases["bm1"][:, :])
            z2p = po.tile([H, G], F32, tag="mm")
            nc.tensor.matmul(z2p[:], lhsT=wm["Wm2"][:], rhs=z1[:],
                             start=True, stop=True)
            z2 = wpool.tile([H, G], BF16, tag="z2")
            nc.scalar.activation(z2[:], z2p[:],
                                 mybir.ActivationFunctionType.Relu,
                                 bias=biases["bm2"][:, :])
            z3p = po.tile([C, G], F32, tag="mm")
            nc.tensor.matmul(z3p[:], lhsT=wm3[:], rhs=z2[:],
                             start=True, stop=True)
            z3 = wpool.tile([C, G], F32, tag="z3")
            nc.vector.tensor_scalar_add(z3[:], z3p[:], bm3[:, :])
            nc.sync.dma_start(out_ext[:, :], z3[:])

    nc.compile()
    return nc


# ---------------------------------------------------------------------------
# entry point
# ---------------------------------------------------------------------------

_CACHE = {}


def kernel(x, src, dst, graph_ids, W_in, b_in, W1, b1, W2, b2,
           Wm1, bm1, Wm2, bm2, Wm3, bm3):
    x = np.asarray(x)
    key = (int(np.asarray(src).sum()) ^ int(np.asarray(dst).sum()),
           int(np.asarray(graph_ids).sum()))
    if key not in _CACHE:
        plan = _plan(src, dst, graph_ids)
        nc = _build(plan)
        _CACHE[key] = (plan, nc)
    plan, nc = _CACHE[key]

    xT = np.concatenate([np.asarray(x).T, np.ones((1, N), np.float32)], axis=0)
    xT = xT[:, plan["perm"]]
    w65 = np.concatenate([np.asarray(W_in), np.asarray(b_in)[None, :]], axis=0)

    def bf(a):
        return np.ascontiguousarray(np.asarray(a), dtype=np.float32).astype(bfloat16)

    def col(a):
        return np.ascontiguousarray(np.asarray(a, np.float32).reshape(-1, 1))

    xTb = bf(xT)
    in_maps = []
    for c in range(CORES):
        in_maps.append({
            "xTs": np.ascontiguousarray(xTb[:, c * SHARD:(c + 1) * SHARD]),
            "W65": bf(w65),
            "Wl1": bf(W1),
            "Wl2": bf(W2),
            "Wm1": bf(Wm1),
            "Wm2": bf(Wm2),
            "Wm3": bf(Wm3),
            "B1": col(np.asarray(b1, np.float32).sum(axis=0)),
            "B2": col(np.asarray(b2, np.float32).sum(axis=0)),
            "bm1": col(bm1),
            "bm2": col(bm2),
            "bm3": col(bm3),
            "IDX": np.ascontiguousarray(plan["IDX"][c]),
            "DSL": np.ascontiguousarray(plan["DSL"][c]),
            "WC": np.ascontiguousarray(plan["WC"][c]),
            "M": np.ascontiguousarray(plan["M"][c]),
        })

    res = bass_utils.run_bass_kernel_spmd(nc, in_maps, list(range(CORES)))
    global LAST_EXEC_NS
    LAST_EXEC_NS = res.exec_time_ns
    out = np.asarray(res.results[0]["out"], np.float32)  # [C, G]
    return np.ascontiguousarray(out.T)                   # [G, C]


LAST_EXEC_NS = None


if __name__ == "__main__":
    import reference
    import jax
    with jax.default_device(jax.devices("cpu")[0]):
        inp = {k: np.asarray(v) for k, v in reference.setup_inputs().items()}
        exp = np.asarray(reference.reference(**{k: v for k, v in inp.items()}))
    act = kernel(**inp)
    rel = np.linalg.norm(act - exp) / np.linalg.norm(exp)
    print("Relative error:", rel)
